# revision 47
# baseline (speedup 1.0000x reference)
"""Binarized LeNet5+BN forward on 8 Trainium2 NeuronCores.

Strategy (data-parallel over batch, 1024 images/core), v2:
  * Feature-major on-chip layout; every layer = matmul-accumulate into PSUM
    followed by ONE activation op (fused conv-bias+BN+hardtanh+binarize).
  * All conv/fc1 matmul operands fp8e4 with perf_mode=DoubleRow (2 K-tiles
    per N=512 pass); host-built Toeplitz +-1/0 weights.
  * v2 pass-count cuts vs v1 (260 -> 217 passes/chunk):
    - conv1 16-feature remainders grouped: 8 y1-rows' remainders packed
      block-diagonally into ONE PSUM bank covering a 384-input-row window
      (1 DR + 1 plain pass per group of 8, x3 groups) instead of 24 passes.
    - conv2 Mt=2 (64-feature) output tiles merged across adjacent y2 pairs
      into full 128-wide passes (4 passes/pair vs 6).
    - fc1 contracts 50 full 128-row act2 tiles = 25 DR passes (vs 30).
  * Activation work split across BOTH ScalarE and VectorE (v1: scalar-only
    at 67% busy was near-critical):
    - Scalar (AF.Sign, +-1 fp8): conv1 mains+remainders, conv2-Mt2-merged,
      fc1, fc2 outputs.
    - Vector (tensor_scalar is_ge, {0,1} fp8): conv2 Mt0/Mt1 outputs.
      Consumers fold the {0,1} encoding: fc1 weight rows for those features
      are 2*w*sign(s2), and the -sum(w*d) constant folds into fc1's Sign
      bias (b3' = b3 - s3*c3). Exact in fp8.
  * Weights packed into few DMA loads issued after the first input tiles;
    HAM warm-up burst before conv1; double-buffered pools throughout.
"""

from contextlib import ExitStack

import ml_dtypes
import numpy as np

import concourse.bacc as bacc
import concourse.tile as tile
from concourse import mybir
from concourse.bass_utils import run_bass_kernel_spmd

F32 = mybir.dt.float32
BF16 = mybir.dt.bfloat16
FP8 = mybir.dt.float8e4
DR = mybir.MatmulPerfMode.DoubleRow
AF = mybir.ActivationFunctionType
GE = mybir.AluOpType.is_ge
EPS = np.float32(1e-5)
N_CORES = 8
B_TOTAL = 8192
B_CORE = B_TOTAL // N_CORES
CHUNK = 512
N_CHUNKS = B_CORE // CHUNK

_f8 = lambda a: np.ascontiguousarray(a.astype(ml_dtypes.float8_e4m3fn))
_bf = lambda a: np.ascontiguousarray(a.astype(ml_dtypes.bfloat16))
_f32c = lambda a: np.ascontiguousarray(a.astype(np.float32))

# wpack column offsets (fp8 [128, 5504]); split for early conv1 load.
# conv2 Mt0/Mt1 weights come in even/odd-y2 variants: odd actc slots are
# DVE-produced {0,1} so their rows carry 2*d1 scaling (see build_consts).
_WOFF = {"w1p0": 0, "w1rA": 256, "w1rB": 1024,
         "w201e": 1408, "w201o": 1920, "w223e": 2432, "w223o": 2944,
         "w24re": 3456, "w24ro": 3968, "wm2": 4480, "wr2m": 5248}
WPACK_COLS = 5504
WSPLIT = 1408  # conv1 weights end
N_WARM = 30  # HAM warm-up matmul count (cold burst until the first slots land)


def _sign(a):
    return np.sign(a).astype(np.float32)


def _toeplitz1(w1s):  # [6,1,5,5] -> [160,144] rows (ky, xi<32), cols (c1,xo)
    W = np.zeros((160, 144), np.float32)
    xo = np.arange(24)
    for ky in range(5):
        for kx in range(5):
            for c1 in range(6):
                W[ky * 32 + xo + kx, c1 * 24 + xo] = w1s[c1, 0, ky, kx]
    return W


def _toeplitz2(w2s):
    """[16,6,5,5] -> main [128,5,320] rows (c1,xi24 mod 128), cols (c2,xo);
    remainder (last 16 rows of each 144-block) at 16-stride: [128,320]."""
    W = np.zeros((720, 320), np.float32)
    xo = np.arange(20)
    for ky in range(5):
        for c1 in range(6):
            for kx in range(5):
                for c2 in range(16):
                    W[ky * 144 + c1 * 24 + xo + kx, c2 * 20 + xo] = w2s[c2, c1, ky, kx]
    main = np.stack([W[144 * k : 144 * k + 128] for k in range(5)], 1)  # [128,5,320]
    rem16 = np.zeros((128, 320), np.float32)  # rows 16k+r (k<5)
    for k in range(5):
        rem16[16 * k : 16 * k + 16] = W[144 * k + 128 : 144 * k + 144]
    return main, rem16


def _affine(g, b, m, v, extra_bias):
    inv = (g.astype(np.float32) / np.sqrt(v.astype(np.float32) + EPS)).astype(np.float32)
    return inv, (inv * (extra_bias.astype(np.float32) - m.astype(np.float32)) + b.astype(np.float32)).astype(np.float32)


def _pair(a, b):  # [128, M] + [<=128, M] -> [128, 2M] interleaved pair-major
    out = np.zeros((128, 2, a.shape[1]), np.float32)
    out[:, 0, :] = a
    out[0 : b.shape[0], 1, :] = b
    return out.reshape(128, -1)


def build_consts(inp):
    """Host-side preprocessing of all weights/BN params into device constants."""
    C = {}
    W1 = _toeplitz1(_sign(inp["conv1_w"]))
    w2main, w2r16 = _toeplitz2(_sign(inp["conv2_w"]))
    wpack = np.zeros((128, WPACK_COLS), np.float32)

    wpack[:, 0:256] = _pair(W1[0:128, 0:128], W1[128:160, 0:128])
    # conv1 remainder groups: group g covers y1 in [8g, 8g+8); its windows
    # span input rows [256g, 256g+384). Col 16k+j <-> (y1=8g+k, feat 128+j).
    for g in range(3):
        A = np.zeros((256, 128), np.float32)
        Bm = np.zeros((128, 128), np.float32)
        for k in range(8):
            full = np.zeros((384, 16), np.float32)
            full[32 * k : 32 * k + 160, :] = W1[:, 128:144]
            A[:, 16 * k : 16 * k + 16] = full[0:256]
            Bm[:, 16 * k : 16 * k + 16] = full[256:384]
        wpack[:, _WOFF["w1rA"] + 256 * g : _WOFF["w1rA"] + 256 * g + 256] = \
            _pair(A[0:128], A[128:256])
        wpack[:, _WOFF["w1rB"] + 128 * g : _WOFF["w1rB"] + 128 * g + 128] = Bm
    # BN affine folds (needed before conv2 packing for the d1 row scaling)
    s1, b1 = _affine(inp["bn1_g"], inp["bn1_b"], inp["bn1_m"], inp["bn1_v"], inp["conv1_b"])
    s2, b2 = _affine(inp["bn2_g"], inp["bn2_b"], inp["bn2_m"], inp["bn2_v"], inp["conv2_b"])
    s3, b3 = _affine(inp["bnf1_g"], inp["bnf1_b"], inp["bnf1_m"], inp["bnf1_v"], inp["fc1_b"])
    s4, b4 = _affine(inp["bnf2_g"], inp["bnf2_b"], inp["bnf2_m"], inp["bnf2_v"], inp["fc2_b"])
    s5, b5 = _affine(inp["bnf3_g"], inp["bnf3_b"], inp["bnf3_m"], inp["bnf3_v"], inp["fc3_b"])
    c1v = np.arange(144) // 24
    sc1f, bi1f = s1[c1v], b1[c1v]
    c2v = np.arange(320) // 20
    sc2f, bi2f = s2[c2v], b2[c2v]
    # conv1 main slots: even y1 -> ScalarE Sign (+-1); odd y1 -> VectorE is_ge
    # ({0,1}); d1 = flip for negative BN scale on the {0,1} decode.
    d1f = np.where(sc1f[0:128] >= 0, np.float32(1.0), np.float32(-1.0))

    def _rsc(w, par_odd):  # scale rows by 2*d1 when the slot parity is odd
        return (2.0 * d1f)[:, None] * w if par_odd else w

    # conv2 Mt0/Mt1: 3 DR pairs each (ky01, ky23, ky4+rem), e/o y2 variants.
    # Pair elem a of pass j touches slot y2+2j+a -> parity (y2+a) % 2.
    for Mt in range(2):
        ms = slice(128 * Mt, 128 * Mt + 128)
        for v, sfx in ((0, "e"), (1, "o")):
            wpack[:, _WOFF["w201" + sfx] + 256 * Mt : _WOFF["w201" + sfx] + 256 * Mt + 256] = \
                _pair(_rsc(w2main[:, 0, ms], (v + 0) % 2), _rsc(w2main[:, 1, ms], (v + 1) % 2))
            wpack[:, _WOFF["w223" + sfx] + 256 * Mt : _WOFF["w223" + sfx] + 256 * Mt + 256] = \
                _pair(_rsc(w2main[:, 2, ms], (v + 0) % 2), _rsc(w2main[:, 3, ms], (v + 1) % 2))
            wpack[:, _WOFF["w24r" + sfx] + 256 * Mt : _WOFF["w24r" + sfx] + 256 * Mt + 256] = \
                _pair(_rsc(w2main[:, 4, ms], (v + 0) % 2), w2r16[:, ms])
    # conv2 Mt2 merged across adjacent (ya, yb=ya+1): cols 0:64 <- ya feats
    # 256:320, cols 64:128 <- yb. Main pass j contracts slots (ya+2j, ya+2j+1);
    # elem a parity = a (ya even).
    m2 = slice(256, 320)
    for j in range(3):
        blk = np.zeros((128, 2, 128), np.float32)
        for a in range(2):
            so = 2 * j + a  # slot offset rel. ya
            if so <= 4:
                blk[:, a, 0:64] = _rsc(w2main[:, so, m2], a % 2)
            if 0 <= so - 1 <= 4:
                blk[:, a, 64:128] = _rsc(w2main[:, so - 1, m2], a % 2)
        wpack[:, _WOFF["wm2"] + 256 * j : _WOFF["wm2"] + 256 * j + 256] = \
            blk.reshape(128, 256)
    blk = np.zeros((128, 2, 128), np.float32)
    blk[:, 0, 0:64] = w2r16[:, m2]   # pair elem 0 = ya's rem bundle slot
    blk[:, 1, 64:128] = w2r16[:, m2]  # pair elem 1 = yb's rem bundle slot
    wpack[:, _WOFF["wr2m"] : _WOFF["wr2m"] + 256] = blk.reshape(128, 256)

    C["wpack1"] = _f8(wpack[:, 0:WSPLIT])
    C["wpack2"] = _f8(wpack[:, WSPLIT:WPACK_COLS])

    # conv2 fold constants: for y2 of parity v, the odd slots in its window
    # contribute -sum(W2*d1): even y2 -> kys {1,3}; odd y2 -> kys {0,2,4}.
    cf_e = ((w2main[:, 1, :] + w2main[:, 3, :]) * d1f[:, None]).sum(0)
    cf_o = ((w2main[:, 0, :] + w2main[:, 2, :] + w2main[:, 4, :]) * d1f[:, None]).sum(0)

    # fc1, permuted to on-chip act2 layout [128, 50, 128] (50 full K-tiles):
    # pair-group p (ya=2p, yb=2p+1) owns blocks 5p..5p+4:
    #   5p+0: ya feats 0:128 ({0,1} DVE) | 5p+1: yb feats 0:128
    #   5p+2: ya feats 128:256           | 5p+3: yb feats 128:256
    #   5p+4: [ya feats 256:320 | yb feats 256:320] (+-1 scalar)
    # {0,1} rows get w'' = 2*d*w (d = sign(s2) flip); fold c3 into fc1 bias.
    w3s = _sign(inp["fc1_w"])  # [120, 6400]
    d2f = np.where(sc2f >= 0, np.float32(1.0), np.float32(-1.0))

    def cols(y2, m):
        return (m // 20) * 400 + y2 * 20 + (m % 20)

    W3T = np.zeros((128, 50, 128), np.float32)
    for p in range(10):
        ya, yb = 2 * p, 2 * p + 1
        m0 = np.arange(128)
        m1 = np.arange(128) + 128
        mm2 = np.arange(64) + 256
        W3T[:, 5 * p + 0, 0:120] = (2 * d2f[m0])[:, None] * w3s[:, cols(ya, m0)].T
        W3T[:, 5 * p + 1, 0:120] = (2 * d2f[m0])[:, None] * w3s[:, cols(yb, m0)].T
        W3T[:, 5 * p + 2, 0:120] = (2 * d2f[m1])[:, None] * w3s[:, cols(ya, m1)].T
        W3T[:, 5 * p + 3, 0:120] = (2 * d2f[m1])[:, None] * w3s[:, cols(yb, m1)].T
        W3T[0:64, 5 * p + 4, 0:120] = w3s[:, cols(ya, mm2)].T
        W3T[64:128, 5 * p + 4, 0:120] = w3s[:, cols(yb, mm2)].T
    C["w3t"] = _f8(W3T)
    # fold constant: c3[m] = sum over {0,1}-encoded inputs of w3s*d
    mdv = np.arange(256)
    c3 = np.zeros(120, np.float32)
    for y2 in range(20):
        c3 += (w3s[:, cols(y2, mdv)] * d2f[mdv][None, :]).sum(1)

    w45 = np.zeros((120, 94), np.float32)
    w45[0:120, 0:84] = _sign(inp["fc2_w"]).T
    w45[0:84, 84:94] = _sign(inp["fc3_w"]).T
    C["w45"] = _bf(w45)

    scp = np.zeros((128, 20), np.float32)
    scp[:, 0], scp[:, 1] = sc1f[:128], bi1f[:128]
    for k in range(8):  # remainder scale/bias at 16-stride
        scp[16 * k : 16 * k + 16, 2] = sc1f[128:]
        scp[16 * k : 16 * k + 16, 3] = bi1f[128:]
    scp[:, 4] = -bi1f[0:128] / sc1f[0:128]        # conv1 odd thr (DVE)
    scp[:, 5] = -bi2f[0:128] / sc2f[0:128] + cf_e[0:128]      # Mt0 thr, even y2
    scp[:, 6] = -bi2f[0:128] / sc2f[0:128] + cf_o[0:128]      # Mt0 thr, odd y2
    scp[:, 7] = -bi2f[128:256] / sc2f[128:256] + cf_e[128:256]
    scp[:, 8] = -bi2f[128:256] / sc2f[128:256] + cf_o[128:256]
    scp[0:64, 9], scp[64:128, 9] = sc2f[256:320], sc2f[256:320]
    scp[0:64, 10] = bi2f[256:320] - sc2f[256:320] * cf_e[256:320]
    scp[64:128, 10] = bi2f[256:320] - sc2f[256:320] * cf_o[256:320]
    scp[0:120, 11], scp[0:120, 12] = s3, b3 - s3 * c3
    scp[0:84, 13], scp[0:84, 14] = s4, b4
    scp[0:10, 15], scp[0:10, 16] = s5, b5
    C["scp"] = _f32c(scp)
    return C


def prep_x(x):
    """sign + feature-major layout + 4 phase shifts: [B,1,28,28] -> per-core
    [N_CHUNKS, 4, 128, 7, CHUNK] fp8 (xT row 32y+x = sign(img[y,x]), x<28).
    Chunk-outer so each on-device load is one fully-contiguous transfer."""
    xs = np.sign(x.reshape(B_TOTAL, 28, 28)).astype(np.float32)
    res = []
    for i in range(N_CORES):
        xc = xs[i * B_CORE : (i + 1) * B_CORE]  # [b, 28, 28]
        tmp = np.zeros((B_CORE, 28, 32), np.float32)
        tmp[:, :, 0:28] = xc
        xT = np.zeros((1024, B_CORE), np.float32)
        xT[0:896] = tmp.reshape(B_CORE, 896).T
        xq = np.stack([xT[32 * q : 32 * q + 896].reshape(7, 128, B_CORE).transpose(1, 0, 2)
                       for q in range(4)])  # [4,128,7,b]
        xqc = xq.reshape(4, 128, 7, N_CHUNKS, CHUNK).transpose(3, 0, 1, 2, 4)
        res.append(_f8(np.ascontiguousarray(xqc)))
    return res


def build_nc(consts, b_core=B_CORE, chunk=CHUNK, stage=99):
    n_chunks = b_core // chunk
    assert chunk % 128 == 0
    nc = bacc.Bacc(None, target_bir_lowering=False, debug=False)
    xt_in = nc.declare_dram_parameter("xt", [n_chunks, 4, 128, 7, chunk], FP8, isOutput=False)
    if stage >= 37:
        out = nc.declare_dram_parameter("out", [10, b_core], F32, isOutput=True)
    else:
        dbg = nc.declare_dram_parameter("dbg", [128, 512], F32, isOutput=True)
    dr = {k: nc.inline_tensor(v, name=f"c_{k}") for k, v in consts.items()}

    with tile.TileContext(nc) as tc, ExitStack() as ctx:
        cp = ctx.enter_context(tc.tile_pool(name="consts", bufs=1))
        xtpool = ctx.enter_context(tc.tile_pool(name="xtpool", bufs=2))
        # PSUM: cps 3 bufs x 2 banks (conv1/conv2 pair tiles, depth-2 act
        # pipelining) + psm 2 bufs x 1 bank (every [*,512] f32 single-bank
        # tile: warmup, conv1 rem, conv2-Mt2m, fc1/fc2/fc3) = 8 banks.
        cps = ctx.enter_context(tc.tile_pool(name="cps", bufs=3, space="PSUM"))
        psm = ctx.enter_context(tc.tile_pool(name="psm", bufs=2, space="PSUM"))
        apool = ctx.enter_context(tc.tile_pool(name="apool", bufs=2))
        a2pool = ctx.enter_context(tc.tile_pool(name="a2pool", bufs=2))
        fpool = ctx.enter_context(tc.tile_pool(name="fpool", bufs=2))
        dpool = ctx.enter_context(tc.tile_pool(name="dpool", bufs=2))

        def load_x(c):
            xtq = [xtpool.tile([128, 7, chunk], FP8, tag=f"xt{q}", name=f"xt{q}")
                   for q in range(4)]
            for q in range(4):
                nc.sync.dma_start(out=xtq[q][:], in_=xt_in[c, q])
            return xtq

        def cload(name, shape, dtype=FP8):
            t = cp.tile(shape, dtype, tag=f"c_{name}", name=f"c_{name}")
            nc.sync.dma_start(out=t[:], in_=dr[name][:])
            return t

        # issue order: first input slots -> conv1 weights/scales -> the rest
        xtq_next = [xtpool.tile([128, 7, chunk], FP8, tag=f"xt{q}", name=f"xt{q}")
                    for q in range(4)]
        # chunk-0 loads in conv1 need-order: group g needs input slots
        # (g, g+1) of all 4 phases + the conv1 weight block, so the first
        # matmuls can start after ~700KB instead of the full ~3.2MB.
        wp = cp.tile([128, WPACK_COLS], FP8, tag="c_wpack", name="c_wpack")
        for q in range(4):
            nc.sync.dma_start(out=xtq_next[q][:, 0:2, :], in_=xt_in[0, q, :, 0:2, :])
        nc.sync.dma_start(out=wp[:, 0:WSPLIT], in_=dr["wpack1"][:])
        scp = cload("scp", [128, 20], F32)
        for q in range(4):
            nc.sync.dma_start(out=xtq_next[q][:, 2:4, :], in_=xt_in[0, q, :, 2:4, :])
        nc.sync.dma_start(out=wp[:, WSPLIT:WPACK_COLS], in_=dr["wpack2"][:])
        for q in range(4):
            nc.sync.dma_start(out=xtq_next[q][:, 4:7, :], in_=xt_in[0, q, :, 4:7, :])
        w3t = cload("w3t", [128, 50, 128])
        w45 = cload("w45", [120, 94], BF16)

        # HAM warm-up burst: dep-free matmuls fill the input-DMA shadow so the
        # PE clock reaches 2.4 GHz before conv1's first real matmul. vector
        # memset, not gpsimd (gpsimd's first op pays a ~6us IRAM load that
        # would delay the whole burst). Dummy activations pull the one-time
        # ACT_TABLE_LOAD (~1.3us) into the warm-up shadow.
        wub = cp.tile([128, 128], BF16, tag="warm")
        nc.vector.memset(wub[:], 1.0)
        dmt = cp.tile([128, 2], BF16, tag="dmt")
        nc.scalar.activation(dmt[:, 0:1], wub[:, 0:1], AF.Sign)
        nc.scalar.activation(dmt[:, 1:2], wub[:, 0:1], AF.Identity)
        f1w = psm.tile([128, CHUNK], F32, tag="sm")
        for _ in range(N_WARM):
            nc.tensor.matmul(f1w[:, 0:128], wub[:], wub[:], start=True, stop=True)

        def wdr(name, Mt=None, g=None, j=None):  # DoubleRow pair view [128, 2, 128]
            o = _WOFF[name]
            if Mt is not None:
                o += 256 * Mt
            if g is not None:
                o += 256 * g
            if j is not None:
                o += 256 * j
            return wp[:, o : o + 256].rearrange("p (a m) -> p a m", a=2)

        def scb(col, p):  # (scale, bias) column pair from scp
            return scp[0:p, col : col + 1], scp[0:p, col + 1 : col + 2]

        # fc2/fc3 of chunk c are software-pipelined into chunk c+1's conv1 so
        # the a3->fc2->a4->fc3 serial act chain hides under real matmuls.
        pend = None  # a3 tile of the previous chunk

        def emit_fc2(a3):
            f2ps = psm.tile([84, chunk], F32, tag="sm")
            nc.tensor.matmul(f2ps[:], w45[0:120, 0:84], a3[:], start=True, stop=True)
            return f2ps

        def emit_a4(f2ps):
            a4 = fpool.tile([84, chunk], BF16, tag="a4")
            s4_, b4_ = scb(13, 84)
            nc.scalar.activation(a4[:], f2ps[:], AF.Sign, bias=b4_, scale=s4_)
            return a4

        def emit_fc3(a4, cc):
            f3ps = psm.tile([10, chunk], F32, tag="sm")
            nc.tensor.matmul(f3ps[:], w45[0:84, 84:94], a4[:], start=True, stop=True)
            o5 = fpool.tile([10, chunk], F32, tag="o5")
            s5_, b5_ = scb(15, 10)
            nc.scalar.activation(o5[:], f3ps[:], AF.Identity, bias=b5_, scale=s5_)
            nc.sync.dma_start(out=out[:, cc * chunk : (cc + 1) * chunk], in_=o5[:])

        for c in range(n_chunks):
            xtq = xtq_next
            if stage <= 1:
                dt_ = dpool.tile([128, 512], F32, tag="dbg")
                nc.vector.tensor_copy(out=dt_[:], in_=xtq[1][:, 0, 0:512])
                nc.sync.dma_start(out=dbg[:], in_=dt_[:])
                continue

            # ---- conv1: 1 DoubleRow matmul per (y1, main); remainders of 8 y1
            # grouped block-diagonally into one PSUM bank (1 DR + 1 plain pass
            # per group), output at 16-feature stride = bundle layout.
            # actc slots: 0..23 main y rows (even: scalar +-1, odd: DVE {0,1});
            # 24+3p+s = remainder bundles (+-1). Each yga group packs its two
            # even y1 in one PSUM pair-tile and its two odd y1 in the other, so
            # each engine runs ONE [128,2,512] act per group (strided output).
            actc = apool.tile([128, 48, chunk], FP8, tag="actc")
            act2 = a2pool.tile([128, 50, chunk], FP8, tag="act2")

            def c2pair(Mt, ya, yb):
                # conv2 Mt0/Mt1 pair (ya, yb) of equal parity: 6 DR passes +
                # one DVE is_ge writing act2 slots (5*(y//2)+2Mt+par).
                par = ya % 2
                sfx = "o" if par else "e"
                ps = cps.tile([128, 2, chunk], F32, tag="cps", name=f"c2ps{Mt}")
                for ty, y2 in ((0, ya), (1, yb)):
                    p8, s8 = y2 % 8, y2 // 8
                    d = (24 + 3 * p8 + s8) - (y2 + 4)
                    nc.tensor.matmul(ps[:, ty, :], wdr("w201" + sfx, Mt=Mt),
                                     actc[:, y2 : y2 + 2, :],
                                     start=True, stop=False, perf_mode=DR)
                    nc.tensor.matmul(ps[:, ty, :], wdr("w223" + sfx, Mt=Mt),
                                     actc[:, y2 + 2 : y2 + 4, :],
                                     start=False, stop=False, perf_mode=DR)
                    nc.tensor.matmul(ps[:, ty, :], wdr("w24r" + sfx, Mt=Mt),
                                     actc[:, y2 + 4 : y2 + 5 + d : d, :],
                                     start=False, stop=True, perf_mode=DR)
                sa = 5 * (ya // 2) + 2 * Mt + par
                sb = 5 * (yb // 2) + 2 * Mt + par
                nc.vector.tensor_scalar(act2[:, sa : sb + 1 : sb - sa, :], ps[:],
                                        scp[0:128, 5 + 2 * Mt + par : 6 + 2 * Mt + par],
                                        None, GE)

            # conv2 pairs (y, y+8): pair 0 needs no bundle phase-copies, pair k
            # needs copy k -- ordered so the interleave below never waits.
            PAIRS2 = [(0, 8), (1, 9), (2, 10), (3, 11), (4, 12),
                      (5, 13), (6, 14), (7, 15), (16, 18), (17, 19)]
            # conv1 groups with conv2-Mt0 pairs interleaved once enough actc
            # slots exist: the PE rides conv2 passes while conv1's act chain
            # (the latency-bound part) drains in the background.
            ILV = {3: [0], 4: [1, 2, 3], 5: [4, 5, 6, 7, 8, 9]}
            f2p_t = a4_t = None
            for gi, yga in enumerate(range(0, 12, 2)):
                ps0s = []
                for par in (0, 1):  # evens tile, odds tile
                    ps0 = cps.tile([128, 2, chunk], F32, tag="cps")
                    ps0s.append(ps0)
                    for ty in range(2):
                        y1 = 2 * yga + par + 2 * ty
                        q, t = y1 % 4, y1 // 4
                        nc.tensor.matmul(ps0[:, ty, :], wdr("w1p0"), xtq[q][:, t : t + 2, :],
                                         start=True, stop=True, perf_mode=DR)
                if gi <= 2:  # remainder group g: 2 passes
                    g = gi
                    c1r = psm.tile([128, chunk], F32, tag="sm")
                    nc.tensor.matmul(c1r[:], wdr("w1rA", g=g), xtq[0][:, 2 * g : 2 * g + 2, :],
                                     start=True, stop=False, perf_mode=DR)
                    nc.tensor.matmul(c1r[:], wp[:, _WOFF["w1rB"] + 128 * g : _WOFF["w1rB"] + 128 * g + 128],
                                     xtq[0][:, 2 * g + 2, :], start=False, stop=True)
                if pend is not None and gi == 0:
                    f2p_t = emit_fc2(pend)
                if pend is not None and gi == 2:
                    emit_fc3(a4_t, c - 1)
                    pend = None


                s0, b0 = scb(0, 128)
                nc.scalar.activation(actc[:, 2 * yga : 2 * yga + 3 : 2, :], ps0s[0][:],
                                     AF.Sign, bias=b0, scale=s0)
                nc.vector.tensor_scalar(actc[:, 2 * yga + 1 : 2 * yga + 4 : 2, :],
                                        ps0s[1][:], scp[0:128, 4:5], None, GE)
                if gi <= 2:
                    s1_, b1_ = scb(2, 128)
                    nc.scalar.activation(actc[:, 24 + gi, :], c1r[:], AF.Sign,
                                         bias=b1_, scale=s1_)
                if f2p_t is not None and gi == 1:
                    a4_t = emit_a4(f2p_t)
                    f2p_t = None
                if gi == 2:
                    # 7 phase-shifted bundle copies (16-row shifts); all rem
                    # signs just landed, so the DMAs start ASAP.
                    for p in range(1, 8):
                        ns = 3 if p <= 3 else 2
                        if p <= 3:
                            nc.vector.memset(actc[:, 24 + 3 * p + 2, :], 0.0)
                        nc.sync.dma_start(out=actc[0 : 128 - 16 * p, 24 + 3 * p : 24 + 3 * p + ns, :],
                                          in_=actc[16 * p : 128, 24 : 24 + ns, :])
                        nc.sync.dma_start(out=actc[128 - 16 * p : 128, 24 + 3 * p : 24 + 3 * p + 2, :],
                                          in_=actc[0 : 16 * p, 25 : 27, :])
                    if c + 1 < n_chunks:  # issue next chunk's input loads
                        xtq_next = load_x(c + 1)
                if stage >= 3:
                    for pi in ILV.get(gi, []):
                        c2pair(0, *PAIRS2[pi])
            if stage <= 2:
                dt_ = dpool.tile([128, 512], F32, tag="dbg")
                nc.vector.tensor_copy(out=dt_[:], in_=actc[:, 0, 0:512])
                nc.sync.dma_start(out=dbg[:], in_=dt_[:])
                continue

            # ---- conv2 Mt1 (Mt0 ran interleaved above) ----
            for ya, yb in PAIRS2:
                c2pair(1, ya, yb)
            for p in range(10):  # Mt2 merged: 4 DR passes per pair; scalar act
                ya = 2 * p
                ps = psm.tile([128, chunk], F32, tag="sm", name="m2ps")
                for j in range(3):
                    nc.tensor.matmul(ps[:], wdr("wm2", j=j),
                                     actc[:, ya + 2 * j : ya + 2 * j + 2, :],
                                     start=(j == 0), stop=False, perf_mode=DR)
                sa = 24 + 3 * (ya % 8) + ya // 8
                nc.tensor.matmul(ps[:], wdr("wr2m"), actc[:, sa : sa + 4 : 3, :],
                                 start=False, stop=True, perf_mode=DR)
                s2_, b2_ = scb(9, 128)
                nc.scalar.activation(act2[:, 5 * p + 4, :], ps[:], AF.Sign,
                                     bias=b2_, scale=s2_)
            if stage <= 3:
                dt_ = dpool.tile([128, 512], F32, tag="dbg")
                nc.vector.tensor_copy(out=dt_[:], in_=act2[:, 0, 0:512])
                nc.sync.dma_start(out=dbg[:], in_=dt_[:])
                continue

            # ---- fc1: 25 DR passes over 50 full K-tiles ----
            f1ps = psm.tile([128, chunk], F32, tag="sm", name="f1ps")
            k = 0
            for p in range(10):
                for off in (0, 2):
                    b = 5 * p + off
                    nc.tensor.matmul(f1ps[:], w3t[:, b : b + 2, :], act2[:, b : b + 2, :],
                                     start=(k == 0), stop=False, perf_mode=DR)
                    k += 1
            for q in range(5):
                b = 10 * q + 4
                nc.tensor.matmul(f1ps[:], w3t[:, b : b + 6 : 5, :], act2[:, b : b + 6 : 5, :],
                                 start=False, stop=(q == 4), perf_mode=DR)
            if stage <= 35:
                a3 = fpool.tile([120, chunk], BF16, tag="a3")
                s3_, b3_ = scb(11, 120)
                nc.scalar.activation(a3[:], f1ps[0:120, :], AF.Sign, bias=b3_, scale=s3_)
                dt_ = dpool.tile([128, 512], F32, tag="dbg")
                nc.any.memset(dt_[:], 0.0)
                nc.vector.tensor_copy(out=dt_[0:120, :], in_=a3[:, 0:512])
                nc.sync.dma_start(out=dbg[:], in_=dt_[:])
                continue
            if c < n_chunks - 1:
                a3 = fpool.tile([120, chunk], BF16, tag="a3")
                s3_, b3_ = scb(11, 120)
                nc.scalar.activation(a3[:], f1ps[0:120, :], AF.Sign, bias=b3_, scale=s3_)
                pend = a3  # fc2/fc3 pipelined into the next chunk
            else:
                pend_ps = f1ps  # epilogue runs the whole tail, split in halves

        if stage >= 37:
            # epilogue for the last chunk: fc1-act..out in two half-batches so
            # the serial act chain pipelines across Scalar/PE/DVE.
            cc = n_chunks - 1
            H = chunk // 2
            a3 = fpool.tile([120, chunk], BF16, tag="a3")
            a4 = fpool.tile([84, chunk], BF16, tag="a4")
            f2ps = psm.tile([84, chunk], F32, tag="sm")
            f3ps = psm.tile([10, chunk], F32, tag="sm")
            o5 = fpool.tile([10, chunk], F32, tag="o5")
            s3_, b3_ = scb(11, 120)
            s4_, b4_ = scb(13, 84)
            s5_, b5_ = scb(15, 10)
            sls = [slice(0, H), slice(H, 2 * H)]
            for sl in sls:
                nc.scalar.activation(a3[:, sl], pend_ps[0:120, sl], AF.Sign,
                                     bias=b3_, scale=s3_)
            for sl in sls:
                nc.tensor.matmul(f2ps[:, sl], w45[0:120, 0:84], a3[:, sl],
                                 start=True, stop=True)
            for sl in sls:
                nc.scalar.activation(a4[:, sl], f2ps[:, sl], AF.Sign,
                                     bias=b4_, scale=s4_)
            for h, sl in enumerate(sls):
                nc.tensor.matmul(f3ps[:, sl], w45[0:84, 84:94], a4[:, sl],
                                 start=True, stop=True)
                nc.vector.tensor_scalar(o5[:, sl], f3ps[:, sl], s5_, b5_,
                                        mybir.AluOpType.mult, mybir.AluOpType.add)
                nc.sync.dma_start(out=out[:, cc * chunk + H * h : cc * chunk + H * (h + 1)],
                                  in_=o5[:, sl])

    nc.compile()
    return nc


def kernel(**inputs):
    inputs = {k: np.asarray(v) for k, v in inputs.items()}
    consts = build_consts(inputs)
    nc = build_nc(consts)
    xs = prep_x(inputs["x"].astype(np.float32))
    in_maps = [{"xt": xs[i]} for i in range(N_CORES)]
    res = run_bass_kernel_spmd(nc, in_maps, core_ids=list(range(N_CORES)))
    out = np.concatenate([np.asarray(r["out"]).astype(np.float32).T for r in res.results], axis=0)
    return out.astype(np.float32)


# revision 48
# speedup vs baseline: 1.0027x; 1.0027x over previous
"""Binarized LeNet5+BN forward on 8 Trainium2 NeuronCores.

Strategy (data-parallel over batch, 1024 images/core), v2:
  * Feature-major on-chip layout; every layer = matmul-accumulate into PSUM
    followed by ONE activation op (fused conv-bias+BN+hardtanh+binarize).
  * All conv/fc1 matmul operands fp8e4 with perf_mode=DoubleRow (2 K-tiles
    per N=512 pass); host-built Toeplitz +-1/0 weights.
  * v2 pass-count cuts vs v1 (260 -> 217 passes/chunk):
    - conv1 16-feature remainders grouped: 8 y1-rows' remainders packed
      block-diagonally into ONE PSUM bank covering a 384-input-row window
      (1 DR + 1 plain pass per group of 8, x3 groups) instead of 24 passes.
    - conv2 Mt=2 (64-feature) output tiles merged across adjacent y2 pairs
      into full 128-wide passes (4 passes/pair vs 6).
    - fc1 contracts 50 full 128-row act2 tiles = 25 DR passes (vs 30).
  * Activation work split across BOTH ScalarE and VectorE (v1: scalar-only
    at 67% busy was near-critical):
    - Scalar (AF.Sign, +-1 fp8): conv1 mains+remainders, conv2-Mt2-merged,
      fc1, fc2 outputs.
    - Vector (tensor_scalar is_ge, {0,1} fp8): conv2 Mt0/Mt1 outputs.
      Consumers fold the {0,1} encoding: fc1 weight rows for those features
      are 2*w*sign(s2), and the -sum(w*d) constant folds into fc1's Sign
      bias (b3' = b3 - s3*c3). Exact in fp8.
  * Weights packed into few DMA loads issued after the first input tiles;
    HAM warm-up burst before conv1; double-buffered pools throughout.
"""

from contextlib import ExitStack

import ml_dtypes
import numpy as np

import concourse.bacc as bacc
import concourse.tile as tile
from concourse import mybir
from concourse.bass_utils import run_bass_kernel_spmd

F32 = mybir.dt.float32
BF16 = mybir.dt.bfloat16
FP8 = mybir.dt.float8e4
DR = mybir.MatmulPerfMode.DoubleRow
AF = mybir.ActivationFunctionType
GE = mybir.AluOpType.is_ge
EPS = np.float32(1e-5)
N_CORES = 8
B_TOTAL = 8192
B_CORE = B_TOTAL // N_CORES
CHUNK = 512
N_CHUNKS = B_CORE // CHUNK

_f8 = lambda a: np.ascontiguousarray(a.astype(ml_dtypes.float8_e4m3fn))
_bf = lambda a: np.ascontiguousarray(a.astype(ml_dtypes.bfloat16))
_f32c = lambda a: np.ascontiguousarray(a.astype(np.float32))

# wpack column offsets (fp8 [128, 5504]); split for early conv1 load.
# conv2 Mt0/Mt1 weights come in even/odd-y2 variants: odd actc slots are
# DVE-produced {0,1} so their rows carry 2*d1 scaling (see build_consts).
_WOFF = {"w1p0": 0, "w1rA": 256, "w1rB": 1024,
         "w201e": 1408, "w201o": 1920, "w223e": 2432, "w223o": 2944,
         "w24re": 3456, "w24ro": 3968, "wm2": 4480, "wr2m": 5248}
WPACK_COLS = 5504
WSPLIT = 1408  # conv1 weights end
N_WARM = 48  # HAM warm-up matmul count (cold burst until the first slots land)


def _sign(a):
    return np.sign(a).astype(np.float32)


def _toeplitz1(w1s):  # [6,1,5,5] -> [160,144] rows (ky, xi<32), cols (c1,xo)
    W = np.zeros((160, 144), np.float32)
    xo = np.arange(24)
    for ky in range(5):
        for kx in range(5):
            for c1 in range(6):
                W[ky * 32 + xo + kx, c1 * 24 + xo] = w1s[c1, 0, ky, kx]
    return W


def _toeplitz2(w2s):
    """[16,6,5,5] -> main [128,5,320] rows (c1,xi24 mod 128), cols (c2,xo);
    remainder (last 16 rows of each 144-block) at 16-stride: [128,320]."""
    W = np.zeros((720, 320), np.float32)
    xo = np.arange(20)
    for ky in range(5):
        for c1 in range(6):
            for kx in range(5):
                for c2 in range(16):
                    W[ky * 144 + c1 * 24 + xo + kx, c2 * 20 + xo] = w2s[c2, c1, ky, kx]
    main = np.stack([W[144 * k : 144 * k + 128] for k in range(5)], 1)  # [128,5,320]
    rem16 = np.zeros((128, 320), np.float32)  # rows 16k+r (k<5)
    for k in range(5):
        rem16[16 * k : 16 * k + 16] = W[144 * k + 128 : 144 * k + 144]
    return main, rem16


def _affine(g, b, m, v, extra_bias):
    inv = (g.astype(np.float32) / np.sqrt(v.astype(np.float32) + EPS)).astype(np.float32)
    return inv, (inv * (extra_bias.astype(np.float32) - m.astype(np.float32)) + b.astype(np.float32)).astype(np.float32)


def _pair(a, b):  # [128, M] + [<=128, M] -> [128, 2M] interleaved pair-major
    out = np.zeros((128, 2, a.shape[1]), np.float32)
    out[:, 0, :] = a
    out[0 : b.shape[0], 1, :] = b
    return out.reshape(128, -1)


def build_consts(inp):
    """Host-side preprocessing of all weights/BN params into device constants."""
    C = {}
    W1 = _toeplitz1(_sign(inp["conv1_w"]))
    w2main, w2r16 = _toeplitz2(_sign(inp["conv2_w"]))
    wpack = np.zeros((128, WPACK_COLS), np.float32)

    wpack[:, 0:256] = _pair(W1[0:128, 0:128], W1[128:160, 0:128])
    # conv1 remainder groups: group g covers y1 in [8g, 8g+8); its windows
    # span input rows [256g, 256g+384). Col 16k+j <-> (y1=8g+k, feat 128+j).
    for g in range(3):
        A = np.zeros((256, 128), np.float32)
        Bm = np.zeros((128, 128), np.float32)
        for k in range(8):
            full = np.zeros((384, 16), np.float32)
            full[32 * k : 32 * k + 160, :] = W1[:, 128:144]
            A[:, 16 * k : 16 * k + 16] = full[0:256]
            Bm[:, 16 * k : 16 * k + 16] = full[256:384]
        wpack[:, _WOFF["w1rA"] + 256 * g : _WOFF["w1rA"] + 256 * g + 256] = \
            _pair(A[0:128], A[128:256])
        wpack[:, _WOFF["w1rB"] + 128 * g : _WOFF["w1rB"] + 128 * g + 128] = Bm
    # BN affine folds (needed before conv2 packing for the d1 row scaling)
    s1, b1 = _affine(inp["bn1_g"], inp["bn1_b"], inp["bn1_m"], inp["bn1_v"], inp["conv1_b"])
    s2, b2 = _affine(inp["bn2_g"], inp["bn2_b"], inp["bn2_m"], inp["bn2_v"], inp["conv2_b"])
    s3, b3 = _affine(inp["bnf1_g"], inp["bnf1_b"], inp["bnf1_m"], inp["bnf1_v"], inp["fc1_b"])
    s4, b4 = _affine(inp["bnf2_g"], inp["bnf2_b"], inp["bnf2_m"], inp["bnf2_v"], inp["fc2_b"])
    s5, b5 = _affine(inp["bnf3_g"], inp["bnf3_b"], inp["bnf3_m"], inp["bnf3_v"], inp["fc3_b"])
    c1v = np.arange(144) // 24
    sc1f, bi1f = s1[c1v], b1[c1v]
    c2v = np.arange(320) // 20
    sc2f, bi2f = s2[c2v], b2[c2v]
    # conv1 main slots: even y1 -> ScalarE Sign (+-1); odd y1 -> VectorE is_ge
    # ({0,1}); d1 = flip for negative BN scale on the {0,1} decode.
    d1f = np.where(sc1f[0:128] >= 0, np.float32(1.0), np.float32(-1.0))

    def _rsc(w, par_odd):  # scale rows by 2*d1 when the slot parity is odd
        return (2.0 * d1f)[:, None] * w if par_odd else w

    # conv2 Mt0/Mt1: 3 DR pairs each (ky01, ky23, ky4+rem), e/o y2 variants.
    # Pair elem a of pass j touches slot y2+2j+a -> parity (y2+a) % 2.
    for Mt in range(2):
        ms = slice(128 * Mt, 128 * Mt + 128)
        for v, sfx in ((0, "e"), (1, "o")):
            wpack[:, _WOFF["w201" + sfx] + 256 * Mt : _WOFF["w201" + sfx] + 256 * Mt + 256] = \
                _pair(_rsc(w2main[:, 0, ms], (v + 0) % 2), _rsc(w2main[:, 1, ms], (v + 1) % 2))
            wpack[:, _WOFF["w223" + sfx] + 256 * Mt : _WOFF["w223" + sfx] + 256 * Mt + 256] = \
                _pair(_rsc(w2main[:, 2, ms], (v + 0) % 2), _rsc(w2main[:, 3, ms], (v + 1) % 2))
            wpack[:, _WOFF["w24r" + sfx] + 256 * Mt : _WOFF["w24r" + sfx] + 256 * Mt + 256] = \
                _pair(_rsc(w2main[:, 4, ms], (v + 0) % 2), w2r16[:, ms])
    # conv2 Mt2 merged across adjacent (ya, yb=ya+1): cols 0:64 <- ya feats
    # 256:320, cols 64:128 <- yb. Main pass j contracts slots (ya+2j, ya+2j+1);
    # elem a parity = a (ya even).
    m2 = slice(256, 320)
    for j in range(3):
        blk = np.zeros((128, 2, 128), np.float32)
        for a in range(2):
            so = 2 * j + a  # slot offset rel. ya
            if so <= 4:
                blk[:, a, 0:64] = _rsc(w2main[:, so, m2], a % 2)
            if 0 <= so - 1 <= 4:
                blk[:, a, 64:128] = _rsc(w2main[:, so - 1, m2], a % 2)
        wpack[:, _WOFF["wm2"] + 256 * j : _WOFF["wm2"] + 256 * j + 256] = \
            blk.reshape(128, 256)
    blk = np.zeros((128, 2, 128), np.float32)
    blk[:, 0, 0:64] = w2r16[:, m2]   # pair elem 0 = ya's rem bundle slot
    blk[:, 1, 64:128] = w2r16[:, m2]  # pair elem 1 = yb's rem bundle slot
    wpack[:, _WOFF["wr2m"] : _WOFF["wr2m"] + 256] = blk.reshape(128, 256)

    C["wpack1"] = _f8(wpack[:, 0:WSPLIT])
    C["wpack2"] = _f8(wpack[:, WSPLIT:WPACK_COLS])

    # conv2 fold constants: for y2 of parity v, the odd slots in its window
    # contribute -sum(W2*d1): even y2 -> kys {1,3}; odd y2 -> kys {0,2,4}.
    cf_e = ((w2main[:, 1, :] + w2main[:, 3, :]) * d1f[:, None]).sum(0)
    cf_o = ((w2main[:, 0, :] + w2main[:, 2, :] + w2main[:, 4, :]) * d1f[:, None]).sum(0)

    # fc1, permuted to on-chip act2 layout [128, 50, 128] (50 full K-tiles):
    # pair-group p (ya=2p, yb=2p+1) owns blocks 5p..5p+4:
    #   5p+0: ya feats 0:128 ({0,1} DVE) | 5p+1: yb feats 0:128
    #   5p+2: ya feats 128:256           | 5p+3: yb feats 128:256
    #   5p+4: [ya feats 256:320 | yb feats 256:320] (+-1 scalar)
    # {0,1} rows get w'' = 2*d*w (d = sign(s2) flip); fold c3 into fc1 bias.
    w3s = _sign(inp["fc1_w"])  # [120, 6400]
    d2f = np.where(sc2f >= 0, np.float32(1.0), np.float32(-1.0))

    def cols(y2, m):
        return (m // 20) * 400 + y2 * 20 + (m % 20)

    W3T = np.zeros((128, 50, 128), np.float32)
    for p in range(10):
        ya, yb = 2 * p, 2 * p + 1
        m0 = np.arange(128)
        m1 = np.arange(128) + 128
        mm2 = np.arange(64) + 256
        W3T[:, 5 * p + 0, 0:120] = (2 * d2f[m0])[:, None] * w3s[:, cols(ya, m0)].T
        W3T[:, 5 * p + 1, 0:120] = (2 * d2f[m0])[:, None] * w3s[:, cols(yb, m0)].T
        W3T[:, 5 * p + 2, 0:120] = (2 * d2f[m1])[:, None] * w3s[:, cols(ya, m1)].T
        W3T[:, 5 * p + 3, 0:120] = (2 * d2f[m1])[:, None] * w3s[:, cols(yb, m1)].T
        W3T[0:64, 5 * p + 4, 0:120] = w3s[:, cols(ya, mm2)].T
        W3T[64:128, 5 * p + 4, 0:120] = w3s[:, cols(yb, mm2)].T
    C["w3t"] = _f8(W3T)
    # fold constant: c3[m] = sum over {0,1}-encoded inputs of w3s*d
    mdv = np.arange(256)
    c3 = np.zeros(120, np.float32)
    for y2 in range(20):
        c3 += (w3s[:, cols(y2, mdv)] * d2f[mdv][None, :]).sum(1)

    w45 = np.zeros((120, 94), np.float32)
    w45[0:120, 0:84] = _sign(inp["fc2_w"]).T
    w45[0:84, 84:94] = _sign(inp["fc3_w"]).T
    C["w45"] = _bf(w45)

    scp = np.zeros((128, 20), np.float32)
    scp[:, 0], scp[:, 1] = sc1f[:128], bi1f[:128]
    for k in range(8):  # remainder scale/bias at 16-stride
        scp[16 * k : 16 * k + 16, 2] = sc1f[128:]
        scp[16 * k : 16 * k + 16, 3] = bi1f[128:]
    scp[:, 4] = -bi1f[0:128] / sc1f[0:128]        # conv1 odd thr (DVE)
    scp[:, 5] = -bi2f[0:128] / sc2f[0:128] + cf_e[0:128]      # Mt0 thr, even y2
    scp[:, 6] = -bi2f[0:128] / sc2f[0:128] + cf_o[0:128]      # Mt0 thr, odd y2
    scp[:, 7] = -bi2f[128:256] / sc2f[128:256] + cf_e[128:256]
    scp[:, 8] = -bi2f[128:256] / sc2f[128:256] + cf_o[128:256]
    scp[0:64, 9], scp[64:128, 9] = sc2f[256:320], sc2f[256:320]
    scp[0:64, 10] = bi2f[256:320] - sc2f[256:320] * cf_e[256:320]
    scp[64:128, 10] = bi2f[256:320] - sc2f[256:320] * cf_o[256:320]
    scp[0:120, 11], scp[0:120, 12] = s3, b3 - s3 * c3
    scp[0:84, 13], scp[0:84, 14] = s4, b4
    scp[0:10, 15], scp[0:10, 16] = s5, b5
    C["scp"] = _f32c(scp)
    return C


def prep_x(x):
    """sign + feature-major layout + 4 phase shifts: [B,1,28,28] -> per-core
    [N_CHUNKS, 4, 128, 7, CHUNK] fp8 (xT row 32y+x = sign(img[y,x]), x<28).
    Chunk-outer so each on-device load is one fully-contiguous transfer."""
    xs = np.sign(x.reshape(B_TOTAL, 28, 28)).astype(np.float32)
    res = []
    for i in range(N_CORES):
        xc = xs[i * B_CORE : (i + 1) * B_CORE]  # [b, 28, 28]
        tmp = np.zeros((B_CORE, 28, 32), np.float32)
        tmp[:, :, 0:28] = xc
        xT = np.zeros((1024, B_CORE), np.float32)
        xT[0:896] = tmp.reshape(B_CORE, 896).T
        xq = np.stack([xT[32 * q : 32 * q + 896].reshape(7, 128, B_CORE).transpose(1, 0, 2)
                       for q in range(4)])  # [4,128,7,b]
        xqc = xq.reshape(4, 128, 7, N_CHUNKS, CHUNK).transpose(3, 0, 1, 2, 4)
        res.append(_f8(np.ascontiguousarray(xqc)))
    return res


def build_nc(consts, b_core=B_CORE, chunk=CHUNK, stage=99):
    n_chunks = b_core // chunk
    assert chunk % 128 == 0
    nc = bacc.Bacc(None, target_bir_lowering=False, debug=False)
    xt_in = nc.declare_dram_parameter("xt", [n_chunks, 4, 128, 7, chunk], FP8, isOutput=False)
    if stage >= 37:
        out = nc.declare_dram_parameter("out", [10, b_core], F32, isOutput=True)
    else:
        dbg = nc.declare_dram_parameter("dbg", [128, 512], F32, isOutput=True)
    dr = {k: nc.inline_tensor(v, name=f"c_{k}") for k, v in consts.items()}

    with tile.TileContext(nc) as tc, ExitStack() as ctx:
        cp = ctx.enter_context(tc.tile_pool(name="consts", bufs=1))
        xtpool = ctx.enter_context(tc.tile_pool(name="xtpool", bufs=2))
        # PSUM: cps 3 bufs x 2 banks (conv1/conv2 pair tiles, depth-2 act
        # pipelining) + psm 2 bufs x 1 bank (every [*,512] f32 single-bank
        # tile: warmup, conv1 rem, conv2-Mt2m, fc1/fc2/fc3) = 8 banks.
        cps = ctx.enter_context(tc.tile_pool(name="cps", bufs=3, space="PSUM"))
        psm = ctx.enter_context(tc.tile_pool(name="psm", bufs=2, space="PSUM"))
        apool = ctx.enter_context(tc.tile_pool(name="apool", bufs=2))
        a2pool = ctx.enter_context(tc.tile_pool(name="a2pool", bufs=2))
        fpool = ctx.enter_context(tc.tile_pool(name="fpool", bufs=2))
        dpool = ctx.enter_context(tc.tile_pool(name="dpool", bufs=2))

        def load_x(c):
            xtq = [xtpool.tile([128, 7, chunk], FP8, tag=f"xt{q}", name=f"xt{q}")
                   for q in range(4)]
            for q in range(4):
                nc.sync.dma_start(out=xtq[q][:], in_=xt_in[c, q])
            return xtq

        def cload(name, shape, dtype=FP8):
            t = cp.tile(shape, dtype, tag=f"c_{name}", name=f"c_{name}")
            nc.sync.dma_start(out=t[:], in_=dr[name][:])
            return t

        # issue order: first input slots -> conv1 weights/scales -> the rest
        xtq_next = [xtpool.tile([128, 7, chunk], FP8, tag=f"xt{q}", name=f"xt{q}")
                    for q in range(4)]
        # chunk-0 loads in conv1 need-order: group g needs input slots
        # (g, g+1) of all 4 phases + the conv1 weight block, so the first
        # matmuls can start after ~700KB instead of the full ~3.2MB.
        wp = cp.tile([128, WPACK_COLS], FP8, tag="c_wpack", name="c_wpack")
        for q in range(4):
            nc.sync.dma_start(out=xtq_next[q][:, 0:2, :], in_=xt_in[0, q, :, 0:2, :])
        nc.sync.dma_start(out=wp[:, 0:WSPLIT], in_=dr["wpack1"][:])
        scp = cload("scp", [128, 20], F32)
        for q in range(4):
            nc.sync.dma_start(out=xtq_next[q][:, 2:4, :], in_=xt_in[0, q, :, 2:4, :])
        nc.sync.dma_start(out=wp[:, WSPLIT:WPACK_COLS], in_=dr["wpack2"][:])
        for q in range(4):
            nc.sync.dma_start(out=xtq_next[q][:, 4:7, :], in_=xt_in[0, q, :, 4:7, :])
        w3t = cload("w3t", [128, 50, 128])
        w45 = cload("w45", [120, 94], BF16)

        # HAM warm-up burst: dep-free matmuls fill the input-DMA shadow so the
        # PE clock reaches 2.4 GHz before conv1's first real matmul. vector
        # memset, not gpsimd (gpsimd's first op pays a ~6us IRAM load that
        # would delay the whole burst). Dummy activations pull the one-time
        # ACT_TABLE_LOAD (~1.3us) into the warm-up shadow.
        wub = cp.tile([128, 128], BF16, tag="warm")
        nc.vector.memset(wub[:], 1.0)
        dmt = cp.tile([128, 2], BF16, tag="dmt")
        nc.scalar.activation(dmt[:, 0:1], wub[:, 0:1], AF.Sign)
        nc.scalar.activation(dmt[:, 1:2], wub[:, 0:1], AF.Identity)
        f1w = psm.tile([128, CHUNK], F32, tag="sm")
        for _ in range(N_WARM):
            nc.tensor.matmul(f1w[:, 0:128], wub[:], wub[:], start=True, stop=True)

        def wdr(name, Mt=None, g=None, j=None):  # DoubleRow pair view [128, 2, 128]
            o = _WOFF[name]
            if Mt is not None:
                o += 256 * Mt
            if g is not None:
                o += 256 * g
            if j is not None:
                o += 256 * j
            return wp[:, o : o + 256].rearrange("p (a m) -> p a m", a=2)

        def scb(col, p):  # (scale, bias) column pair from scp
            return scp[0:p, col : col + 1], scp[0:p, col + 1 : col + 2]

        # fc2/fc3 of chunk c are software-pipelined into chunk c+1's conv1 so
        # the a3->fc2->a4->fc3 serial act chain hides under real matmuls.
        pend = None  # a3 tile of the previous chunk

        def emit_fc2(a3):
            f2ps = psm.tile([84, chunk], F32, tag="sm")
            nc.tensor.matmul(f2ps[:], w45[0:120, 0:84], a3[:], start=True, stop=True)
            return f2ps

        def emit_a4(f2ps):
            a4 = fpool.tile([84, chunk], BF16, tag="a4")
            s4_, b4_ = scb(13, 84)
            nc.scalar.activation(a4[:], f2ps[:], AF.Sign, bias=b4_, scale=s4_)
            return a4

        def emit_fc3(a4, cc):
            f3ps = psm.tile([10, chunk], F32, tag="sm")
            nc.tensor.matmul(f3ps[:], w45[0:84, 84:94], a4[:], start=True, stop=True)
            o5 = fpool.tile([10, chunk], F32, tag="o5")
            s5_, b5_ = scb(15, 10)
            nc.scalar.activation(o5[:], f3ps[:], AF.Identity, bias=b5_, scale=s5_)
            nc.sync.dma_start(out=out[:, cc * chunk : (cc + 1) * chunk], in_=o5[:])

        for c in range(n_chunks):
            xtq = xtq_next
            if stage <= 1:
                dt_ = dpool.tile([128, 512], F32, tag="dbg")
                nc.vector.tensor_copy(out=dt_[:], in_=xtq[1][:, 0, 0:512])
                nc.sync.dma_start(out=dbg[:], in_=dt_[:])
                continue

            # ---- conv1: 1 DoubleRow matmul per (y1, main); remainders of 8 y1
            # grouped block-diagonally into one PSUM bank (1 DR + 1 plain pass
            # per group), output at 16-feature stride = bundle layout.
            # actc slots: 0..23 main y rows (even: scalar +-1, odd: DVE {0,1});
            # 24+3p+s = remainder bundles (+-1). Each yga group packs its two
            # even y1 in one PSUM pair-tile and its two odd y1 in the other, so
            # each engine runs ONE [128,2,512] act per group (strided output).
            actc = apool.tile([128, 48, chunk], FP8, tag="actc")
            act2 = a2pool.tile([128, 50, chunk], FP8, tag="act2")

            def c2pair(Mt, ya, yb):
                # conv2 Mt0/Mt1 pair (ya, yb) of equal parity: 6 DR passes +
                # one DVE is_ge writing act2 slots (5*(y//2)+2Mt+par).
                par = ya % 2
                sfx = "o" if par else "e"
                ps = cps.tile([128, 2, chunk], F32, tag="cps", name=f"c2ps{Mt}")
                for ty, y2 in ((0, ya), (1, yb)):
                    p8, s8 = y2 % 8, y2 // 8
                    d = (24 + 3 * p8 + s8) - (y2 + 4)
                    nc.tensor.matmul(ps[:, ty, :], wdr("w201" + sfx, Mt=Mt),
                                     actc[:, y2 : y2 + 2, :],
                                     start=True, stop=False, perf_mode=DR)
                    nc.tensor.matmul(ps[:, ty, :], wdr("w223" + sfx, Mt=Mt),
                                     actc[:, y2 + 2 : y2 + 4, :],
                                     start=False, stop=False, perf_mode=DR)
                    nc.tensor.matmul(ps[:, ty, :], wdr("w24r" + sfx, Mt=Mt),
                                     actc[:, y2 + 4 : y2 + 5 + d : d, :],
                                     start=False, stop=True, perf_mode=DR)
                sa = 5 * (ya // 2) + 2 * Mt + par
                sb = 5 * (yb // 2) + 2 * Mt + par
                nc.vector.tensor_scalar(act2[:, sa : sb + 1 : sb - sa, :], ps[:],
                                        scp[0:128, 5 + 2 * Mt + par : 6 + 2 * Mt + par],
                                        None, GE)

            # conv2 pairs (y, y+8): pair 0 needs no bundle phase-copies, pair k
            # needs copy k -- ordered so the interleave below never waits.
            PAIRS2 = [(0, 8), (1, 9), (2, 10), (3, 11), (4, 12),
                      (5, 13), (6, 14), (7, 15), (16, 18), (17, 19)]
            # conv1 groups with conv2-Mt0 pairs interleaved once enough actc
            # slots exist: the PE rides conv2 passes while conv1's act chain
            # (the latency-bound part) drains in the background.
            ILV = {3: [0], 4: [1, 2, 3], 5: [4, 5, 6, 7, 8, 9]}
            f2p_t = a4_t = None
            for gi, yga in enumerate(range(0, 12, 2)):
                ps0s = []
                for par in (0, 1):  # evens tile, odds tile
                    ps0 = cps.tile([128, 2, chunk], F32, tag="cps")
                    ps0s.append(ps0)
                    for ty in range(2):
                        y1 = 2 * yga + par + 2 * ty
                        q, t = y1 % 4, y1 // 4
                        nc.tensor.matmul(ps0[:, ty, :], wdr("w1p0"), xtq[q][:, t : t + 2, :],
                                         start=True, stop=True, perf_mode=DR)
                if gi <= 2:  # remainder group g: 2 passes
                    g = gi
                    c1r = psm.tile([128, chunk], F32, tag="sm")
                    nc.tensor.matmul(c1r[:], wdr("w1rA", g=g), xtq[0][:, 2 * g : 2 * g + 2, :],
                                     start=True, stop=False, perf_mode=DR)
                    nc.tensor.matmul(c1r[:], wp[:, _WOFF["w1rB"] + 128 * g : _WOFF["w1rB"] + 128 * g + 128],
                                     xtq[0][:, 2 * g + 2, :], start=False, stop=True)
                if pend is not None and gi == 0:
                    f2p_t = emit_fc2(pend)
                if pend is not None and gi == 2:
                    emit_fc3(a4_t, c - 1)
                    pend = None


                s0, b0 = scb(0, 128)
                nc.scalar.activation(actc[:, 2 * yga : 2 * yga + 3 : 2, :], ps0s[0][:],
                                     AF.Sign, bias=b0, scale=s0)
                nc.vector.tensor_scalar(actc[:, 2 * yga + 1 : 2 * yga + 4 : 2, :],
                                        ps0s[1][:], scp[0:128, 4:5], None, GE)
                if gi <= 2:
                    s1_, b1_ = scb(2, 128)
                    nc.scalar.activation(actc[:, 24 + gi, :], c1r[:], AF.Sign,
                                         bias=b1_, scale=s1_)
                if f2p_t is not None and gi == 1:
                    a4_t = emit_a4(f2p_t)
                    f2p_t = None
                if gi == 2:
                    # 7 phase-shifted bundle copies (16-row shifts); all rem
                    # signs just landed, so the DMAs start ASAP.
                    for p in range(1, 8):
                        ns = 3 if p <= 3 else 2
                        if p <= 3:
                            nc.vector.memset(actc[:, 24 + 3 * p + 2, :], 0.0)
                        nc.sync.dma_start(out=actc[0 : 128 - 16 * p, 24 + 3 * p : 24 + 3 * p + ns, :],
                                          in_=actc[16 * p : 128, 24 : 24 + ns, :])
                        nc.sync.dma_start(out=actc[128 - 16 * p : 128, 24 + 3 * p : 24 + 3 * p + 2, :],
                                          in_=actc[0 : 16 * p, 25 : 27, :])
                    if c + 1 < n_chunks:  # issue next chunk's input loads
                        xtq_next = load_x(c + 1)
                if stage >= 3:
                    for pi in ILV.get(gi, []):
                        c2pair(0, *PAIRS2[pi])
            if stage <= 2:
                dt_ = dpool.tile([128, 512], F32, tag="dbg")
                nc.vector.tensor_copy(out=dt_[:], in_=actc[:, 0, 0:512])
                nc.sync.dma_start(out=dbg[:], in_=dt_[:])
                continue

            # ---- conv2 Mt1 (Mt0 ran interleaved above) ----
            for ya, yb in PAIRS2:
                c2pair(1, ya, yb)
            for p in range(10):  # Mt2 merged: 4 DR passes per pair; scalar act
                ya = 2 * p
                ps = psm.tile([128, chunk], F32, tag="sm", name="m2ps")
                for j in range(3):
                    nc.tensor.matmul(ps[:], wdr("wm2", j=j),
                                     actc[:, ya + 2 * j : ya + 2 * j + 2, :],
                                     start=(j == 0), stop=False, perf_mode=DR)
                sa = 24 + 3 * (ya % 8) + ya // 8
                nc.tensor.matmul(ps[:], wdr("wr2m"), actc[:, sa : sa + 4 : 3, :],
                                 start=False, stop=True, perf_mode=DR)
                s2_, b2_ = scb(9, 128)
                nc.scalar.activation(act2[:, 5 * p + 4, :], ps[:], AF.Sign,
                                     bias=b2_, scale=s2_)
            if stage <= 3:
                dt_ = dpool.tile([128, 512], F32, tag="dbg")
                nc.vector.tensor_copy(out=dt_[:], in_=act2[:, 0, 0:512])
                nc.sync.dma_start(out=dbg[:], in_=dt_[:])
                continue

            # ---- fc1: 25 DR passes over 50 full K-tiles ----
            f1ps = psm.tile([128, chunk], F32, tag="sm", name="f1ps")
            k = 0
            for p in range(10):
                for off in (0, 2):
                    b = 5 * p + off
                    nc.tensor.matmul(f1ps[:], w3t[:, b : b + 2, :], act2[:, b : b + 2, :],
                                     start=(k == 0), stop=False, perf_mode=DR)
                    k += 1
            for q in range(5):
                b = 10 * q + 4
                nc.tensor.matmul(f1ps[:], w3t[:, b : b + 6 : 5, :], act2[:, b : b + 6 : 5, :],
                                 start=False, stop=(q == 4), perf_mode=DR)
            if stage <= 35:
                a3 = fpool.tile([120, chunk], BF16, tag="a3")
                s3_, b3_ = scb(11, 120)
                nc.scalar.activation(a3[:], f1ps[0:120, :], AF.Sign, bias=b3_, scale=s3_)
                dt_ = dpool.tile([128, 512], F32, tag="dbg")
                nc.any.memset(dt_[:], 0.0)
                nc.vector.tensor_copy(out=dt_[0:120, :], in_=a3[:, 0:512])
                nc.sync.dma_start(out=dbg[:], in_=dt_[:])
                continue
            if c < n_chunks - 1:
                a3 = fpool.tile([120, chunk], BF16, tag="a3")
                s3_, b3_ = scb(11, 120)
                nc.scalar.activation(a3[:], f1ps[0:120, :], AF.Sign, bias=b3_, scale=s3_)
                pend = a3  # fc2/fc3 pipelined into the next chunk
            else:
                pend_ps = f1ps  # epilogue runs the whole tail, split in halves

        if stage >= 37:
            # epilogue for the last chunk: fc1-act..out in two half-batches so
            # the serial act chain pipelines across Scalar/PE/DVE.
            cc = n_chunks - 1
            H = chunk // 2
            a3 = fpool.tile([120, chunk], BF16, tag="a3")
            a4 = fpool.tile([84, chunk], BF16, tag="a4")
            f2ps = psm.tile([84, chunk], F32, tag="sm")
            f3ps = psm.tile([10, chunk], F32, tag="sm")
            o5 = fpool.tile([10, chunk], F32, tag="o5")
            s3_, b3_ = scb(11, 120)
            s4_, b4_ = scb(13, 84)
            s5_, b5_ = scb(15, 10)
            sls = [slice(0, H), slice(H, 2 * H)]
            for sl in sls:
                nc.scalar.activation(a3[:, sl], pend_ps[0:120, sl], AF.Sign,
                                     bias=b3_, scale=s3_)
            for sl in sls:
                nc.tensor.matmul(f2ps[:, sl], w45[0:120, 0:84], a3[:, sl],
                                 start=True, stop=True)
            for sl in sls:
                nc.scalar.activation(a4[:, sl], f2ps[:, sl], AF.Sign,
                                     bias=b4_, scale=s4_)
            for h, sl in enumerate(sls):
                nc.tensor.matmul(f3ps[:, sl], w45[0:84, 84:94], a4[:, sl],
                                 start=True, stop=True)
                nc.vector.tensor_scalar(o5[:, sl], f3ps[:, sl], s5_, b5_,
                                        mybir.AluOpType.mult, mybir.AluOpType.add)
                nc.sync.dma_start(out=out[:, cc * chunk + H * h : cc * chunk + H * (h + 1)],
                                  in_=o5[:, sl])

    nc.compile()
    return nc


def kernel(**inputs):
    inputs = {k: np.asarray(v) for k, v in inputs.items()}
    consts = build_consts(inputs)
    nc = build_nc(consts)
    xs = prep_x(inputs["x"].astype(np.float32))
    in_maps = [{"xt": xs[i]} for i in range(N_CORES)]
    res = run_bass_kernel_spmd(nc, in_maps, core_ids=list(range(N_CORES)))
    out = np.concatenate([np.asarray(r["out"]).astype(np.float32).T for r in res.results], axis=0)
    return out.astype(np.float32)


# revision 50
# speedup vs baseline: 1.0231x; 1.0204x over previous
"""Binarized LeNet5+BN forward on 8 Trainium2 NeuronCores.

Strategy (data-parallel over batch, 1024 images/core), v2:
  * Feature-major on-chip layout; every layer = matmul-accumulate into PSUM
    followed by ONE activation op (fused conv-bias+BN+hardtanh+binarize).
  * All conv/fc1 matmul operands fp8e4 with perf_mode=DoubleRow (2 K-tiles
    per N=512 pass); host-built Toeplitz +-1/0 weights.
  * v2 pass-count cuts vs v1 (260 -> 217 passes/chunk):
    - conv1 16-feature remainders grouped: 8 y1-rows' remainders packed
      block-diagonally into ONE PSUM bank covering a 384-input-row window
      (1 DR + 1 plain pass per group of 8, x3 groups) instead of 24 passes.
    - conv2 Mt=2 (64-feature) output tiles merged across adjacent y2 pairs
      into full 128-wide passes (4 passes/pair vs 6).
    - fc1 contracts 50 full 128-row act2 tiles = 25 DR passes (vs 30).
  * Activation work split across BOTH ScalarE and VectorE (v1: scalar-only
    at 67% busy was near-critical):
    - Scalar (AF.Sign, +-1 fp8): conv1 mains+remainders, conv2-Mt2-merged,
      fc1, fc2 outputs.
    - Vector (tensor_scalar is_ge, {0,1} fp8): conv2 Mt0/Mt1 outputs.
      Consumers fold the {0,1} encoding: fc1 weight rows for those features
      are 2*w*sign(s2), and the -sum(w*d) constant folds into fc1's Sign
      bias (b3' = b3 - s3*c3). Exact in fp8.
  * Weights packed into few DMA loads issued after the first input tiles;
    HAM warm-up burst before conv1; double-buffered pools throughout.
"""

from contextlib import ExitStack

import ml_dtypes
import numpy as np

import concourse.bacc as bacc
import concourse.tile as tile
from concourse import mybir
from concourse.bass_utils import run_bass_kernel_spmd

F32 = mybir.dt.float32
BF16 = mybir.dt.bfloat16
FP8 = mybir.dt.float8e4
DR = mybir.MatmulPerfMode.DoubleRow
AF = mybir.ActivationFunctionType
GE = mybir.AluOpType.is_ge
EPS = np.float32(1e-5)
N_CORES = 8
B_TOTAL = 8192
B_CORE = B_TOTAL // N_CORES
CHUNK = 512
N_CHUNKS = B_CORE // CHUNK

_f8 = lambda a: np.ascontiguousarray(a.astype(ml_dtypes.float8_e4m3fn))
_bf = lambda a: np.ascontiguousarray(a.astype(ml_dtypes.bfloat16))
_f32c = lambda a: np.ascontiguousarray(a.astype(np.float32))

# wpack column offsets (fp8 [128, 5504]); split for early conv1 load.
# conv2 Mt0/Mt1 weights come in even/odd-y2 variants: odd actc slots are
# DVE-produced {0,1} so their rows carry 2*d1 scaling (see build_consts).
_WOFF = {"w1p0": 0, "w1rA": 256, "w1rB": 1024,
         "w201e": 1408, "w201o": 1920, "w223e": 2432, "w223o": 2944,
         "w24re": 3456, "w24ro": 3968, "wm2": 4480, "wr2m": 5248}
WPACK_COLS = 5504
WSPLIT = 1408  # conv1 weights end
N_WARM = 64  # HAM warm-up matmul count (cold burst covering the input DMA)


def _sign(a):
    return np.sign(a).astype(np.float32)


def _toeplitz1(w1s):  # [6,1,5,5] -> [160,144] rows (ky, xi<32), cols (c1,xo)
    W = np.zeros((160, 144), np.float32)
    xo = np.arange(24)
    for ky in range(5):
        for kx in range(5):
            for c1 in range(6):
                W[ky * 32 + xo + kx, c1 * 24 + xo] = w1s[c1, 0, ky, kx]
    return W


def _toeplitz2(w2s):
    """[16,6,5,5] -> main [128,5,320] rows (c1,xi24 mod 128), cols (c2,xo);
    remainder (last 16 rows of each 144-block) at 16-stride: [128,320]."""
    W = np.zeros((720, 320), np.float32)
    xo = np.arange(20)
    for ky in range(5):
        for c1 in range(6):
            for kx in range(5):
                for c2 in range(16):
                    W[ky * 144 + c1 * 24 + xo + kx, c2 * 20 + xo] = w2s[c2, c1, ky, kx]
    main = np.stack([W[144 * k : 144 * k + 128] for k in range(5)], 1)  # [128,5,320]
    rem16 = np.zeros((128, 320), np.float32)  # rows 16k+r (k<5)
    for k in range(5):
        rem16[16 * k : 16 * k + 16] = W[144 * k + 128 : 144 * k + 144]
    return main, rem16


def _affine(g, b, m, v, extra_bias):
    inv = (g.astype(np.float32) / np.sqrt(v.astype(np.float32) + EPS)).astype(np.float32)
    return inv, (inv * (extra_bias.astype(np.float32) - m.astype(np.float32)) + b.astype(np.float32)).astype(np.float32)


def _pair(a, b):  # [128, M] + [<=128, M] -> [128, 2M] interleaved pair-major
    out = np.zeros((128, 2, a.shape[1]), np.float32)
    out[:, 0, :] = a
    out[0 : b.shape[0], 1, :] = b
    return out.reshape(128, -1)


def build_consts(inp):
    """Host-side preprocessing of all weights/BN params into device constants."""
    C = {}
    W1 = _toeplitz1(_sign(inp["conv1_w"]))
    w2main, w2r16 = _toeplitz2(_sign(inp["conv2_w"]))
    wpack = np.zeros((128, WPACK_COLS), np.float32)

    wpack[:, 0:256] = _pair(W1[0:128, 0:128], W1[128:160, 0:128])
    # conv1 remainder groups: group g covers y1 in [8g, 8g+8); its windows
    # span input rows [256g, 256g+384). Col 16k+j <-> (y1=8g+k, feat 128+j).
    for g in range(3):
        A = np.zeros((256, 128), np.float32)
        Bm = np.zeros((128, 128), np.float32)
        for k in range(8):
            full = np.zeros((384, 16), np.float32)
            full[32 * k : 32 * k + 160, :] = W1[:, 128:144]
            A[:, 16 * k : 16 * k + 16] = full[0:256]
            Bm[:, 16 * k : 16 * k + 16] = full[256:384]
        wpack[:, _WOFF["w1rA"] + 256 * g : _WOFF["w1rA"] + 256 * g + 256] = \
            _pair(A[0:128], A[128:256])
        wpack[:, _WOFF["w1rB"] + 128 * g : _WOFF["w1rB"] + 128 * g + 128] = Bm
    # BN affine folds (needed before conv2 packing for the d1 row scaling)
    s1, b1 = _affine(inp["bn1_g"], inp["bn1_b"], inp["bn1_m"], inp["bn1_v"], inp["conv1_b"])
    s2, b2 = _affine(inp["bn2_g"], inp["bn2_b"], inp["bn2_m"], inp["bn2_v"], inp["conv2_b"])
    s3, b3 = _affine(inp["bnf1_g"], inp["bnf1_b"], inp["bnf1_m"], inp["bnf1_v"], inp["fc1_b"])
    s4, b4 = _affine(inp["bnf2_g"], inp["bnf2_b"], inp["bnf2_m"], inp["bnf2_v"], inp["fc2_b"])
    s5, b5 = _affine(inp["bnf3_g"], inp["bnf3_b"], inp["bnf3_m"], inp["bnf3_v"], inp["fc3_b"])
    c1v = np.arange(144) // 24
    sc1f, bi1f = s1[c1v], b1[c1v]
    c2v = np.arange(320) // 20
    sc2f, bi2f = s2[c2v], b2[c2v]
    # conv1 main slots: even y1 -> ScalarE Sign (+-1); odd y1 -> VectorE is_ge
    # ({0,1}); d1 = flip for negative BN scale on the {0,1} decode.
    d1f = np.where(sc1f[0:128] >= 0, np.float32(1.0), np.float32(-1.0))

    def _rsc(w, par_odd):  # scale rows by 2*d1 when the slot parity is odd
        return (2.0 * d1f)[:, None] * w if par_odd else w

    # conv2 Mt0/Mt1: 3 DR pairs each (ky01, ky23, ky4+rem), e/o y2 variants.
    # Pair elem a of pass j touches slot y2+2j+a -> parity (y2+a) % 2.
    for Mt in range(2):
        ms = slice(128 * Mt, 128 * Mt + 128)
        for v, sfx in ((0, "e"), (1, "o")):
            wpack[:, _WOFF["w201" + sfx] + 256 * Mt : _WOFF["w201" + sfx] + 256 * Mt + 256] = \
                _pair(_rsc(w2main[:, 0, ms], (v + 0) % 2), _rsc(w2main[:, 1, ms], (v + 1) % 2))
            wpack[:, _WOFF["w223" + sfx] + 256 * Mt : _WOFF["w223" + sfx] + 256 * Mt + 256] = \
                _pair(_rsc(w2main[:, 2, ms], (v + 0) % 2), _rsc(w2main[:, 3, ms], (v + 1) % 2))
            wpack[:, _WOFF["w24r" + sfx] + 256 * Mt : _WOFF["w24r" + sfx] + 256 * Mt + 256] = \
                _pair(_rsc(w2main[:, 4, ms], (v + 0) % 2), w2r16[:, ms])
    # conv2 Mt2 merged across adjacent (ya, yb=ya+1): cols 0:64 <- ya feats
    # 256:320, cols 64:128 <- yb. Main pass j contracts slots (ya+2j, ya+2j+1);
    # elem a parity = a (ya even).
    m2 = slice(256, 320)
    for j in range(3):
        blk = np.zeros((128, 2, 128), np.float32)
        for a in range(2):
            so = 2 * j + a  # slot offset rel. ya
            if so <= 4:
                blk[:, a, 0:64] = _rsc(w2main[:, so, m2], a % 2)
            if 0 <= so - 1 <= 4:
                blk[:, a, 64:128] = _rsc(w2main[:, so - 1, m2], a % 2)
        wpack[:, _WOFF["wm2"] + 256 * j : _WOFF["wm2"] + 256 * j + 256] = \
            blk.reshape(128, 256)
    blk = np.zeros((128, 2, 128), np.float32)
    blk[:, 0, 0:64] = w2r16[:, m2]   # pair elem 0 = ya's rem bundle slot
    blk[:, 1, 64:128] = w2r16[:, m2]  # pair elem 1 = yb's rem bundle slot
    wpack[:, _WOFF["wr2m"] : _WOFF["wr2m"] + 256] = blk.reshape(128, 256)

    C["wpack1"] = _f8(wpack[:, 0:WSPLIT])
    C["wpack2"] = _f8(wpack[:, WSPLIT:WPACK_COLS])

    # conv2 fold constants: for y2 of parity v, the odd slots in its window
    # contribute -sum(W2*d1): even y2 -> kys {1,3}; odd y2 -> kys {0,2,4}.
    cf_e = ((w2main[:, 1, :] + w2main[:, 3, :]) * d1f[:, None]).sum(0)
    cf_o = ((w2main[:, 0, :] + w2main[:, 2, :] + w2main[:, 4, :]) * d1f[:, None]).sum(0)

    # fc1, permuted to on-chip act2 layout [128, 50, 128] (50 full K-tiles):
    # pair-group p (ya=2p, yb=2p+1) owns blocks 5p..5p+4:
    #   5p+0: ya feats 0:128 ({0,1} DVE) | 5p+1: yb feats 0:128
    #   5p+2: ya feats 128:256           | 5p+3: yb feats 128:256
    #   5p+4: [ya feats 256:320 | yb feats 256:320] (+-1 scalar)
    # {0,1} rows get w'' = 2*d*w (d = sign(s2) flip); fold c3 into fc1 bias.
    w3s = _sign(inp["fc1_w"])  # [120, 6400]
    d2f = np.where(sc2f >= 0, np.float32(1.0), np.float32(-1.0))

    def cols(y2, m):
        return (m // 20) * 400 + y2 * 20 + (m % 20)

    W3T = np.zeros((128, 50, 128), np.float32)
    for p in range(10):
        ya, yb = 2 * p, 2 * p + 1
        m0 = np.arange(128)
        m1 = np.arange(128) + 128
        mm2 = np.arange(64) + 256
        W3T[:, 5 * p + 0, 0:120] = (2 * d2f[m0])[:, None] * w3s[:, cols(ya, m0)].T
        W3T[:, 5 * p + 1, 0:120] = (2 * d2f[m0])[:, None] * w3s[:, cols(yb, m0)].T
        W3T[:, 5 * p + 2, 0:120] = (2 * d2f[m1])[:, None] * w3s[:, cols(ya, m1)].T
        W3T[:, 5 * p + 3, 0:120] = (2 * d2f[m1])[:, None] * w3s[:, cols(yb, m1)].T
        W3T[0:64, 5 * p + 4, 0:120] = w3s[:, cols(ya, mm2)].T
        W3T[64:128, 5 * p + 4, 0:120] = w3s[:, cols(yb, mm2)].T
    C["w3t"] = _f8(W3T)
    # fold constant: c3[m] = sum over {0,1}-encoded inputs of w3s*d
    mdv = np.arange(256)
    c3 = np.zeros(120, np.float32)
    for y2 in range(20):
        c3 += (w3s[:, cols(y2, mdv)] * d2f[mdv][None, :]).sum(1)

    w45 = np.zeros((120, 94), np.float32)
    w45[0:120, 0:84] = _sign(inp["fc2_w"]).T
    w45[0:84, 84:94] = _sign(inp["fc3_w"]).T
    C["w45"] = _bf(w45)

    scp = np.zeros((128, 20), np.float32)
    scp[:, 0], scp[:, 1] = sc1f[:128], bi1f[:128]
    for k in range(8):  # remainder scale/bias at 16-stride
        scp[16 * k : 16 * k + 16, 2] = sc1f[128:]
        scp[16 * k : 16 * k + 16, 3] = bi1f[128:]
    scp[:, 4] = -bi1f[0:128] / sc1f[0:128]        # conv1 odd thr (DVE)
    scp[:, 5] = -bi2f[0:128] / sc2f[0:128] + cf_e[0:128]      # Mt0 thr, even y2
    scp[:, 6] = -bi2f[0:128] / sc2f[0:128] + cf_o[0:128]      # Mt0 thr, odd y2
    scp[:, 7] = -bi2f[128:256] / sc2f[128:256] + cf_e[128:256]
    scp[:, 8] = -bi2f[128:256] / sc2f[128:256] + cf_o[128:256]
    scp[0:64, 9], scp[64:128, 9] = sc2f[256:320], sc2f[256:320]
    scp[0:64, 10] = bi2f[256:320] - sc2f[256:320] * cf_e[256:320]
    scp[64:128, 10] = bi2f[256:320] - sc2f[256:320] * cf_o[256:320]
    scp[0:120, 11], scp[0:120, 12] = s3, b3 - s3 * c3
    scp[0:84, 13], scp[0:84, 14] = s4, b4
    scp[0:10, 15], scp[0:10, 16] = s5, b5
    C["scp"] = _f32c(scp)
    return C


def prep_x(x):
    """sign + feature-major layout + 4 phase shifts: [B,1,28,28] -> per-core
    [N_CHUNKS, 4, 128, 7, CHUNK] fp8 (xT row 32y+x = sign(img[y,x]), x<28).
    Chunk-outer so each on-device load is one fully-contiguous transfer."""
    xs = np.sign(x.reshape(B_TOTAL, 28, 28)).astype(np.float32)
    res = []
    for i in range(N_CORES):
        xc = xs[i * B_CORE : (i + 1) * B_CORE]  # [b, 28, 28]
        tmp = np.zeros((B_CORE, 28, 32), np.float32)
        tmp[:, :, 0:28] = xc
        xT = np.zeros((1024, B_CORE), np.float32)
        xT[0:896] = tmp.reshape(B_CORE, 896).T
        xq = np.stack([xT[32 * q : 32 * q + 896].reshape(7, 128, B_CORE).transpose(1, 0, 2)
                       for q in range(4)])  # [4,128,7,b]
        xqc = xq.reshape(4, 128, 7, N_CHUNKS, CHUNK).transpose(3, 0, 1, 2, 4)
        res.append(_f8(np.ascontiguousarray(xqc)))
    return res


def build_nc(consts, b_core=B_CORE, chunk=CHUNK, stage=99):
    n_chunks = b_core // chunk
    assert chunk % 128 == 0
    nc = bacc.Bacc(None, target_bir_lowering=False, debug=False)
    xt_in = nc.declare_dram_parameter("xt", [n_chunks, 4, 128, 7, chunk], FP8, isOutput=False)
    if stage >= 37:
        out = nc.declare_dram_parameter("out", [10, b_core], F32, isOutput=True)
    else:
        dbg = nc.declare_dram_parameter("dbg", [128, 512], F32, isOutput=True)
    dr = {k: nc.inline_tensor(v, name=f"c_{k}") for k, v in consts.items()}

    with tile.TileContext(nc) as tc, ExitStack() as ctx:
        cp = ctx.enter_context(tc.tile_pool(name="consts", bufs=1))
        xtpool = ctx.enter_context(tc.tile_pool(name="xtpool", bufs=2))
        # PSUM: cps 3 bufs x 2 banks (conv1/conv2 pair tiles, depth-2 act
        # pipelining) + psm 2 bufs x 1 bank (every [*,512] f32 single-bank
        # tile: warmup, conv1 rem, conv2-Mt2m, fc1/fc2/fc3) = 8 banks.
        cps = ctx.enter_context(tc.tile_pool(name="cps", bufs=3, space="PSUM"))
        psm = ctx.enter_context(tc.tile_pool(name="psm", bufs=2, space="PSUM"))
        apool = ctx.enter_context(tc.tile_pool(name="apool", bufs=2))
        a2pool = ctx.enter_context(tc.tile_pool(name="a2pool", bufs=2))
        fpool = ctx.enter_context(tc.tile_pool(name="fpool", bufs=2))
        dpool = ctx.enter_context(tc.tile_pool(name="dpool", bufs=2))

        def load_x(c):
            xtq = [xtpool.tile([128, 7, chunk], FP8, tag=f"xt{q}", name=f"xt{q}")
                   for q in range(4)]
            for q in range(4):
                nc.sync.dma_start(out=xtq[q][:], in_=xt_in[c, q])
            return xtq

        def cload(name, shape, dtype=FP8):
            t = cp.tile(shape, dtype, tag=f"c_{name}", name=f"c_{name}")
            nc.sync.dma_start(out=t[:], in_=dr[name][:])
            return t

        # issue order: first input slots -> conv1 weights/scales -> the rest
        xtq_next = [xtpool.tile([128, 7, chunk], FP8, tag=f"xt{q}", name=f"xt{q}")
                    for q in range(4)]
        wp = cp.tile([128, WPACK_COLS], FP8, tag="c_wpack", name="c_wpack")
        for q in range(4):
            nc.sync.dma_start(out=xtq_next[q][:, 0:3, :], in_=xt_in[0, q, :, 0:3, :])
        nc.sync.dma_start(out=wp[:, 0:WSPLIT], in_=dr["wpack1"][:])
        scp = cload("scp", [128, 20], F32)
        for q in range(4):
            nc.sync.dma_start(out=xtq_next[q][:, 3:7, :], in_=xt_in[0, q, :, 3:7, :])
        nc.sync.dma_start(out=wp[:, WSPLIT:WPACK_COLS], in_=dr["wpack2"][:])
        w3t = cload("w3t", [128, 50, 128])
        w45 = cload("w45", [120, 94], BF16)

        # HAM warm-up burst: dep-free matmuls fill the input-DMA shadow so the
        # PE clock reaches 2.4 GHz before conv1's first real matmul. vector
        # memset, not gpsimd (gpsimd's first op pays a ~6us IRAM load that
        # would delay the whole burst). Dummy activations pull the one-time
        # ACT_TABLE_LOAD (~1.3us) into the warm-up shadow.
        wub = cp.tile([128, 128], BF16, tag="warm")
        nc.vector.memset(wub[:], 1.0)
        dmt = cp.tile([128, 2], BF16, tag="dmt")
        nc.scalar.activation(dmt[:, 0:1], wub[:, 0:1], AF.Sign)
        nc.scalar.activation(dmt[:, 1:2], wub[:, 0:1], AF.Identity)
        f1w = psm.tile([128, CHUNK], F32, tag="sm")
        for _ in range(N_WARM):
            nc.tensor.matmul(f1w[:, 0:128], wub[:], wub[:], start=True, stop=True)

        def wdr(name, Mt=None, g=None, j=None):  # DoubleRow pair view [128, 2, 128]
            o = _WOFF[name]
            if Mt is not None:
                o += 256 * Mt
            if g is not None:
                o += 256 * g
            if j is not None:
                o += 256 * j
            return wp[:, o : o + 256].rearrange("p (a m) -> p a m", a=2)

        def scb(col, p):  # (scale, bias) column pair from scp
            return scp[0:p, col : col + 1], scp[0:p, col + 1 : col + 2]

        # fc2/fc3 of chunk c are software-pipelined into chunk c+1's conv1 so
        # the a3->fc2->a4->fc3 serial act chain hides under real matmuls.
        pend = None  # a3 tile of the previous chunk

        def emit_fc2(a3):
            f2ps = psm.tile([84, chunk], F32, tag="sm")
            nc.tensor.matmul(f2ps[:], w45[0:120, 0:84], a3[:], start=True, stop=True)
            return f2ps

        def emit_a4(f2ps):
            a4 = fpool.tile([84, chunk], BF16, tag="a4")
            s4_, b4_ = scb(13, 84)
            nc.scalar.activation(a4[:], f2ps[:], AF.Sign, bias=b4_, scale=s4_)
            return a4

        def emit_fc3(a4, cc):
            f3ps = psm.tile([10, chunk], F32, tag="sm")
            nc.tensor.matmul(f3ps[:], w45[0:84, 84:94], a4[:], start=True, stop=True)
            o5 = fpool.tile([10, chunk], F32, tag="o5")
            s5_, b5_ = scb(15, 10)
            nc.scalar.activation(o5[:], f3ps[:], AF.Identity, bias=b5_, scale=s5_)
            nc.sync.dma_start(out=out[:, cc * chunk : (cc + 1) * chunk], in_=o5[:])

        for c in range(n_chunks):
            xtq = xtq_next
            if stage <= 1:
                dt_ = dpool.tile([128, 512], F32, tag="dbg")
                nc.vector.tensor_copy(out=dt_[:], in_=xtq[1][:, 0, 0:512])
                nc.sync.dma_start(out=dbg[:], in_=dt_[:])
                continue

            # ---- conv1: 1 DoubleRow matmul per (y1, main); remainders of 8 y1
            # grouped block-diagonally into one PSUM bank (1 DR + 1 plain pass
            # per group), output at 16-feature stride = bundle layout.
            # actc slots: 0..23 main y rows (even: scalar +-1, odd: DVE {0,1});
            # 24+3p+s = remainder bundles (+-1). Each yga group packs its two
            # even y1 in one PSUM pair-tile and its two odd y1 in the other, so
            # each engine runs ONE [128,2,512] act per group (strided output).
            actc = apool.tile([128, 48, chunk], FP8, tag="actc")
            act2 = a2pool.tile([128, 50, chunk], FP8, tag="act2")

            def c2pair(Mt, ya, yb):
                # conv2 Mt0/Mt1 pair (ya, yb) of equal parity: 6 DR passes +
                # one DVE is_ge writing act2 slots (5*(y//2)+2Mt+par).
                par = ya % 2
                sfx = "o" if par else "e"
                ps = cps.tile([128, 2, chunk], F32, tag="cps", name=f"c2ps{Mt}")
                for ty, y2 in ((0, ya), (1, yb)):
                    p8, s8 = y2 % 8, y2 // 8
                    d = (24 + 3 * p8 + s8) - (y2 + 4)
                    nc.tensor.matmul(ps[:, ty, :], wdr("w201" + sfx, Mt=Mt),
                                     actc[:, y2 : y2 + 2, :],
                                     start=True, stop=False, perf_mode=DR)
                    nc.tensor.matmul(ps[:, ty, :], wdr("w223" + sfx, Mt=Mt),
                                     actc[:, y2 + 2 : y2 + 4, :],
                                     start=False, stop=False, perf_mode=DR)
                    nc.tensor.matmul(ps[:, ty, :], wdr("w24r" + sfx, Mt=Mt),
                                     actc[:, y2 + 4 : y2 + 5 + d : d, :],
                                     start=False, stop=True, perf_mode=DR)
                sa = 5 * (ya // 2) + 2 * Mt + par
                sb = 5 * (yb // 2) + 2 * Mt + par
                nc.vector.tensor_scalar(act2[:, sa : sb + 1 : sb - sa, :], ps[:],
                                        scp[0:128, 5 + 2 * Mt + par : 6 + 2 * Mt + par],
                                        None, GE)

            # conv2 pairs (y, y+8): pair 0 needs no bundle phase-copies, pair k
            # needs copy k -- ordered so the interleave below never waits.
            PAIRS2 = [(0, 8), (1, 9), (2, 10), (3, 11), (4, 12),
                      (5, 13), (6, 14), (7, 15), (16, 18), (17, 19)]
            # conv1 groups with conv2-Mt0 pairs interleaved once enough actc
            # slots exist: the PE rides conv2 passes while conv1's act chain
            # (the latency-bound part) drains in the background.
            ILV = {3: [0], 4: [1, 2, 3], 5: [4, 5, 6, 7, 8, 9]}
            f2p_t = a4_t = None
            for gi, yga in enumerate(range(0, 12, 2)):
                ps0s = []
                for par in (0, 1):  # evens tile, odds tile
                    ps0 = cps.tile([128, 2, chunk], F32, tag="cps")
                    ps0s.append(ps0)
                    for ty in range(2):
                        y1 = 2 * yga + par + 2 * ty
                        q, t = y1 % 4, y1 // 4
                        nc.tensor.matmul(ps0[:, ty, :], wdr("w1p0"), xtq[q][:, t : t + 2, :],
                                         start=True, stop=True, perf_mode=DR)
                if gi <= 2:  # remainder group g: 2 passes
                    g = gi
                    c1r = psm.tile([128, chunk], F32, tag="sm")
                    nc.tensor.matmul(c1r[:], wdr("w1rA", g=g), xtq[0][:, 2 * g : 2 * g + 2, :],
                                     start=True, stop=False, perf_mode=DR)
                    nc.tensor.matmul(c1r[:], wp[:, _WOFF["w1rB"] + 128 * g : _WOFF["w1rB"] + 128 * g + 128],
                                     xtq[0][:, 2 * g + 2, :], start=False, stop=True)
                if pend is not None and gi == 0:
                    f2p_t = emit_fc2(pend)
                if pend is not None and gi == 2:
                    emit_fc3(a4_t, c - 1)
                    pend = None


                s0, b0 = scb(0, 128)
                nc.scalar.activation(actc[:, 2 * yga : 2 * yga + 3 : 2, :], ps0s[0][:],
                                     AF.Sign, bias=b0, scale=s0)
                nc.vector.tensor_scalar(actc[:, 2 * yga + 1 : 2 * yga + 4 : 2, :],
                                        ps0s[1][:], scp[0:128, 4:5], None, GE)
                if gi <= 2:
                    s1_, b1_ = scb(2, 128)
                    nc.scalar.activation(actc[:, 24 + gi, :], c1r[:], AF.Sign,
                                         bias=b1_, scale=s1_)
                if f2p_t is not None and gi == 1:
                    a4_t = emit_a4(f2p_t)
                    f2p_t = None
                if gi == 2:
                    # 7 phase-shifted bundle copies (16-row shifts); all rem
                    # signs just landed, so the DMAs start ASAP.
                    for p in range(1, 8):
                        ns = 3 if p <= 3 else 2
                        if p <= 3:
                            nc.vector.memset(actc[:, 24 + 3 * p + 2, :], 0.0)
                        nc.sync.dma_start(out=actc[0 : 128 - 16 * p, 24 + 3 * p : 24 + 3 * p + ns, :],
                                          in_=actc[16 * p : 128, 24 : 24 + ns, :])
                        nc.sync.dma_start(out=actc[128 - 16 * p : 128, 24 + 3 * p : 24 + 3 * p + 2, :],
                                          in_=actc[0 : 16 * p, 25 : 27, :])
                    if c + 1 < n_chunks:  # issue next chunk's input loads
                        xtq_next = load_x(c + 1)
                if stage >= 3:
                    for pi in ILV.get(gi, []):
                        c2pair(0, *PAIRS2[pi])
            if stage <= 2:
                dt_ = dpool.tile([128, 512], F32, tag="dbg")
                nc.vector.tensor_copy(out=dt_[:], in_=actc[:, 0, 0:512])
                nc.sync.dma_start(out=dbg[:], in_=dt_[:])
                continue

            # ---- conv2 Mt1 (Mt0 ran interleaved above) ----
            for ya, yb in PAIRS2:
                c2pair(1, ya, yb)
            for p in range(10):  # Mt2 merged: 4 DR passes per pair; scalar act
                ya = 2 * p
                ps = psm.tile([128, chunk], F32, tag="sm", name="m2ps")
                for j in range(3):
                    nc.tensor.matmul(ps[:], wdr("wm2", j=j),
                                     actc[:, ya + 2 * j : ya + 2 * j + 2, :],
                                     start=(j == 0), stop=False, perf_mode=DR)
                sa = 24 + 3 * (ya % 8) + ya // 8
                nc.tensor.matmul(ps[:], wdr("wr2m"), actc[:, sa : sa + 4 : 3, :],
                                 start=False, stop=True, perf_mode=DR)
                s2_, b2_ = scb(9, 128)
                nc.scalar.activation(act2[:, 5 * p + 4, :], ps[:], AF.Sign,
                                     bias=b2_, scale=s2_)
            if stage <= 3:
                dt_ = dpool.tile([128, 512], F32, tag="dbg")
                nc.vector.tensor_copy(out=dt_[:], in_=act2[:, 0, 0:512])
                nc.sync.dma_start(out=dbg[:], in_=dt_[:])
                continue

            # ---- fc1: 25 DR passes over 50 full K-tiles ----
            f1ps = psm.tile([128, chunk], F32, tag="sm", name="f1ps")
            k = 0
            for p in range(10):
                for off in (0, 2):
                    b = 5 * p + off
                    nc.tensor.matmul(f1ps[:], w3t[:, b : b + 2, :], act2[:, b : b + 2, :],
                                     start=(k == 0), stop=False, perf_mode=DR)
                    k += 1
            for q in range(5):
                b = 10 * q + 4
                nc.tensor.matmul(f1ps[:], w3t[:, b : b + 6 : 5, :], act2[:, b : b + 6 : 5, :],
                                 start=False, stop=(q == 4), perf_mode=DR)
            if stage <= 35:
                a3 = fpool.tile([120, chunk], BF16, tag="a3")
                s3_, b3_ = scb(11, 120)
                nc.scalar.activation(a3[:], f1ps[0:120, :], AF.Sign, bias=b3_, scale=s3_)
                dt_ = dpool.tile([128, 512], F32, tag="dbg")
                nc.any.memset(dt_[:], 0.0)
                nc.vector.tensor_copy(out=dt_[0:120, :], in_=a3[:, 0:512])
                nc.sync.dma_start(out=dbg[:], in_=dt_[:])
                continue
            if c < n_chunks - 1:
                a3 = fpool.tile([120, chunk], BF16, tag="a3")
                s3_, b3_ = scb(11, 120)
                nc.scalar.activation(a3[:], f1ps[0:120, :], AF.Sign, bias=b3_, scale=s3_)
                pend = a3  # fc2/fc3 pipelined into the next chunk
            else:
                pend_ps = f1ps  # epilogue runs the whole tail, split in halves

        if stage >= 37:
            # epilogue for the last chunk: fc1-act..out in two half-batches so
            # the serial act chain pipelines across Scalar/PE/DVE.
            cc = n_chunks - 1
            H = chunk // 2
            a3 = fpool.tile([120, chunk], BF16, tag="a3")
            a4 = fpool.tile([84, chunk], BF16, tag="a4")
            f2ps = psm.tile([84, chunk], F32, tag="sm")
            f3ps = psm.tile([10, chunk], F32, tag="sm")
            o5 = fpool.tile([10, chunk], F32, tag="o5")
            s3_, b3_ = scb(11, 120)
            s4_, b4_ = scb(13, 84)
            s5_, b5_ = scb(15, 10)
            sls = [slice(0, H), slice(H, 2 * H)]
            for sl in sls:
                nc.scalar.activation(a3[:, sl], pend_ps[0:120, sl], AF.Sign,
                                     bias=b3_, scale=s3_)
            for sl in sls:
                nc.tensor.matmul(f2ps[:, sl], w45[0:120, 0:84], a3[:, sl],
                                 start=True, stop=True)
            for sl in sls:
                nc.scalar.activation(a4[:, sl], f2ps[:, sl], AF.Sign,
                                     bias=b4_, scale=s4_)
            for h, sl in enumerate(sls):
                nc.tensor.matmul(f3ps[:, sl], w45[0:84, 84:94], a4[:, sl],
                                 start=True, stop=True)
                nc.vector.tensor_scalar(o5[:, sl], f3ps[:, sl], s5_, b5_,
                                        mybir.AluOpType.mult, mybir.AluOpType.add)
                nc.sync.dma_start(out=out[:, cc * chunk + H * h : cc * chunk + H * (h + 1)],
                                  in_=o5[:, sl])

    nc.compile()
    return nc


def kernel(**inputs):
    inputs = {k: np.asarray(v) for k, v in inputs.items()}
    consts = build_consts(inputs)
    nc = build_nc(consts)
    xs = prep_x(inputs["x"].astype(np.float32))
    in_maps = [{"xt": xs[i]} for i in range(N_CORES)]
    res = run_bass_kernel_spmd(nc, in_maps, core_ids=list(range(N_CORES)))
    out = np.concatenate([np.asarray(r["out"]).astype(np.float32).T for r in res.results], axis=0)
    return out.astype(np.float32)
